# revision 20
# baseline (speedup 1.0000x reference)
"""Trainium2 Bass kernel for nn_ActorCritic loss (8-core SPMD, data-parallel over batch).

Strategy:
- Shard batch M=512 over 8 cores (64 each). MLP params replicated.
- Host prep: features pre-transposed to feature-major [F, T] per core with token
  column t = m*J + u, u = J-1-j (time-reversed so the GAE recursion becomes a
  forward hardware scan). LayerNorm mean-subtraction is folded into the weights
  (W - rowmean, b - mean); LN gain g is folded into the rstd computation via
  exp(-0.5*ln(var+eps) + ln g).
- On chip per core: 3 MLPs (critic/target/actor) in feature-major layout, bf16
  matmuls with fp32 PSUM. Per-token variance via ones-matrix matmul broadcast.
  ELU via exp/min/max with scalar_tensor_tensor fusion. GAE via
  tensor_tensor_scan. Reality weight via suffix-cumsum trick (no reversal).
  Actor head: stacked [4x32, 512] logits tiles, block-diagonal ones matmul for
  per-token reductions over A.
- Each core emits partial loss sums; host combines to the final [2] output.
"""
import os
import sys
import numpy as np

for _p in ("/opt/trn_rl_repo",):
    if _p not in sys.path and os.path.isdir(_p):
        sys.path.insert(0, _p)

import ml_dtypes  # noqa: E402

bf16 = ml_dtypes.bfloat16

GAMMA = 0.999
LAMBDA = 0.95
TEMPERATURE = 0.001
J, M, F, A = 128, 512, 256, 32
H = J - 1
HID, NLAYERS = 400, 4
LN_EPS = 1e-3
NCORES = 8
MC = M // NCORES          # 64 batch elements per core
T = MC * J                # 8192 tokens per core
NG = 4                    # token groups
GW = T // NG              # 2048 cols per group
NT = 4                    # 512-col tiles per group
TW = GW // NT             # 512

CSZ = [128, 128, 128, 16]         # dout chunks of HID=400
COFS = [0, 128, 256, 384]
KSZ_L0 = [128, 128]               # din chunks of F=256
KSZ_H = [128, 128, 128, 16]       # din chunks of HID=400
KSZ_F = [128, 128, 128, 17]       # din chunks of HID+1 (bias row)

_NETS = [("c", 1), ("t", 1), ("a", 32)]


# ----------------------------------------------------------------------------
# host-side prep
# ----------------------------------------------------------------------------

def _prep_net(params):
    hidden, (Wf, bf_) = params
    pre = []
    for (W, b, g, beta) in hidden:
        W = np.asarray(W, np.float32)
        b = np.asarray(b, np.float32)
        g = np.asarray(g, np.float32)
        beta = np.asarray(beta, np.float32)
        Wc = W - W.mean(axis=1, keepdims=True)
        bc = b - b.mean()
        if np.any(g <= 0):
            raise ValueError("LN gain fold requires g > 0")
        pre.append((Wc.astype(bf16), bc, np.log(g), beta))
    Wf = np.asarray(Wf, np.float32)
    bf_ = np.asarray(bf_, np.float32)
    Wf_aug = np.concatenate([Wf, bf_[None, :]], axis=0).astype(bf16)
    return pre, Wf_aug


def _param_tile(pre_l):
    """[128, 12] fp32: col 3c+0 = centered bias, 3c+1 = ln g, 3c+2 = beta for chunk c."""
    _, bc, lng, beta = pre_l
    out = np.zeros((128, 12), np.float32)
    for c in range(4):
        n = CSZ[c]
        sl = slice(COFS[c], COFS[c] + n)
        out[:n, 3 * c + 0] = bc[sl]
        out[:n, 3 * c + 1] = np.exp(lng[sl]) * LN_EPS ** -0.5
        out[:n, 3 * c + 2] = beta[sl]
    return out


def prep_all(features, rewards_mean, terminals_mean, actions,
             actor_params, critic_params, target_params):
    features = np.asarray(features, np.float32)
    rewards_mean = np.asarray(rewards_mean, np.float32)
    terminals_mean = np.asarray(terminals_mean, np.float32)
    actions = np.asarray(actions, np.float32)

    nets = {}
    for key, params in (("c", critic_params), ("t", target_params), ("a", actor_params)):
        nets[key] = _prep_net(params)

    shared = {"bdiag": np.kron(np.eye(4, dtype=np.float32),
                               np.ones((32, 32), np.float32)).astype(bf16)}
    for nk, (pre, Wf_aug) in nets.items():
        shared[f"wf_{nk}"] = np.ascontiguousarray(Wf_aug)
        for l, p in enumerate(pre):
            shared[f"w_{nk}_{l}"] = np.ascontiguousarray(p[0])
            shared[f"p_{nk}_{l}"] = _param_tile(p)

    in_maps = []
    for core in range(NCORES):
        msl = slice(core * MC, (core + 1) * MC)
        d = dict(shared)
        x0 = features[::-1, msl, :].transpose(2, 1, 0).reshape(F, T)
        d["x0"] = np.ascontiguousarray(x0.astype(bf16))
        d["rew"] = np.ascontiguousarray(rewards_mean[::-1, msl].T)
        d["term"] = np.ascontiguousarray(terminals_mean[::-1, msl].T)
        act_u = np.zeros((J, MC, A), np.float32)
        act_u[1:] = actions[::-1, msl, :]
        act_T = act_u.transpose(2, 1, 0).reshape(A, T)          # [32, 8192]
        act_stk = act_T.reshape(A, NG, NT, TW).transpose(1, 2, 0, 3).reshape(NG, 128, TW)
        d["act_stk"] = np.ascontiguousarray(act_stk.astype(bf16))
        in_maps.append(d)
    return in_maps


# ----------------------------------------------------------------------------
# bass graph
# ----------------------------------------------------------------------------

def build_graph():
    import contextlib
    import concourse.bass as bass
    import concourse.tile as tile
    from concourse import bacc, mybir

    f32 = mybir.dt.float32
    b16 = mybir.dt.bfloat16
    AF = mybir.ActivationFunctionType
    OP = mybir.AluOpType

    nc = bacc.Bacc()

    x0_e = nc.declare_dram_parameter("x0", [F, T], b16, isOutput=False)
    rew_e = nc.declare_dram_parameter("rew", [MC, J], f32, isOutput=False)
    term_e = nc.declare_dram_parameter("term", [MC, J], f32, isOutput=False)
    act_e = nc.declare_dram_parameter("act_stk", [NG, 128, TW], b16, isOutput=False)
    bdiag_e = nc.declare_dram_parameter("bdiag", [128, 128], b16, isOutput=False)
    w_e, p_e, wf_e = {}, {}, {}
    for nk, dout in _NETS:
        wf_e[nk] = nc.declare_dram_parameter(f"wf_{nk}", [HID + 1, dout], b16, isOutput=False)
        for l in range(NLAYERS):
            din = F if l == 0 else HID
            w_e[(nk, l)] = nc.declare_dram_parameter(f"w_{nk}_{l}", [din, HID], b16, isOutput=False)
            p_e[(nk, l)] = nc.declare_dram_parameter(f"p_{nk}_{l}", [128, 12], f32, isOutput=False)
    out_e = nc.declare_dram_parameter("out", [128, 8], f32, isOutput=True)

    vd = {nk: nc.dram_tensor(f"vd_{nk}", [NG * NT, TW], f32) for nk in ("c", "t")}
    gbuf = nc.dram_tensor("gbuf", [T], f32)
    rwbuf = nc.dram_tensor("rwbuf", [T], f32)

    with tile.TileContext(nc) as tc, contextlib.ExitStack() as ctx:
        persist = ctx.enter_context(tc.tile_pool(name="persist", bufs=1))
        wpool = ctx.enter_context(tc.tile_pool(name="wpool", bufs=2))
        xpool = ctx.enter_context(tc.tile_pool(name="xpool", bufs=2))
        x0pool = ctx.enter_context(tc.tile_pool(name="x0pool", bufs=2))
        mid = ctx.enter_context(tc.tile_pool(name="mid", bufs=2))
        ctp = ctx.enter_context(tc.tile_pool(name="ctp", bufs=2))
        small = ctx.enter_context(tc.tile_pool(name="small", bufs=1))
        headp = ctx.enter_context(tc.tile_pool(name="headp", bufs=1))
        psum_c = ctx.enter_context(tc.tile_pool(name="psum_c", bufs=4, space="PSUM"))
        psum_s = ctx.enter_context(tc.tile_pool(name="psum_s", bufs=4, space="PSUM"))

        # --- constants ---
        ones_kk = persist.tile([128, 128], b16, tag="ones_kk")
        nc.vector.memset(ones_kk, 1.0)
        bdiag = persist.tile([128, 128], b16, tag="bdiag")
        nc.sync.dma_start(out=bdiag, in_=bdiag_e[:, :])
        rew = persist.tile([MC, J], f32, tag="rew")
        nc.sync.dma_start(out=rew, in_=rew_e[:, :])
        term = persist.tile([MC, J], f32, tag="term")
        nc.sync.dma_start(out=term, in_=term_e[:, :])
        pol_cols = persist.tile([128, NG], f32, tag="pol_cols")
        ent_cols = persist.tile([128, NG], f32, tag="ent_cols")

        ptiles = {}
        for nk, _ in _NETS:
            for l in range(NLAYERS):
                pt = persist.tile([128, 12], f32, tag=f"p_{nk}_{l}")
                nc.sync.dma_start(out=pt, in_=p_e[(nk, l)][:, :])
                pt2 = persist.tile([128, 12], f32, name=f"p2_{nk}_{l}",
                                   tag=f"p2_{nk}_{l}")
                nc.scalar.copy(pt2, pt)
                ptiles[(nk, l)] = pt2

        def load_weights(nk):
            """DMA one net's weights into SBUF chunk tiles."""
            ws = {}
            for l in range(NLAYERS):
                ksz = KSZ_L0 if l == 0 else KSZ_H
                kofs = 0
                for ki, kn in enumerate(ksz):
                    wt = wpool.tile([kn, HID], b16, tag=f"w_l{l}_k{ki}")
                    nc.sync.dma_start(out=wt, in_=w_e[(nk, l)][kofs:kofs + kn, :])
                    ws[(l, ki)] = wt
                    kofs += kn
            dout = dict(_NETS)[nk]
            kofs = 0
            for ki, kn in enumerate(KSZ_F):
                wt = wpool.tile([kn, dout], b16, tag=f"wf_k{ki}")
                nc.sync.dma_start(out=wt, in_=wf_e[nk][kofs:kofs + kn, :])
                ws[("f", ki)] = wt
                kofs += kn
            return ws

        def mlp_group(nk, ws, g):
            """Run 4 hidden layers for token group g. Returns X4 chunk tiles
            ([csz, GW], chunk 3 is [17, GW] with a ones row at 16)."""
            gofs = g * GW
            xin = []
            for ki, kn in enumerate(KSZ_L0):
                xt = x0pool.tile([kn, GW], b16, name=f"x0_{ki}", tag=f"x0_{ki}")
                nc.sync.dma_start(out=xt, in_=x0_e[ki * 128:ki * 128 + kn, gofs:gofs + GW])
                xin.append(xt)

            cur = xin
            for l in range(NLAYERS):
                ksz = KSZ_L0 if l == 0 else KSZ_H
                pt = ptiles[(nk, l)]
                ct = [ctp.tile([CSZ[c], GW], b16, name=f"ct{c}", tag=f"ct{c}")
                      for c in range(4)]
                lnv = mid.tile([128, GW], b16, tag="lnv")
                sqs = {}
                for t in range(NT):
                    tsl = slice(t * TW, (t + 1) * TW)
                    for c in range(4):
                        csz = CSZ[c]
                        pc = psum_c.tile([csz, TW], f32, tag="pc")
                        for ki, kn in enumerate(ksz):
                            nc.tensor.matmul(
                                pc, lhsT=ws[(l, ki)][:, COFS[c]:COFS[c] + csz],
                                rhs=cur[ki][0:kn, tsl],
                                start=(ki == 0), stop=(ki == len(ksz) - 1))
                        # PSUM -> SBUF cast with centered-bias add; split ACT/DVE
                        dst = ct[c][:, tsl]
                        if c < 2:
                            nc.scalar.activation(dst, pc, AF.Identity,
                                                 bias=pt[0:csz, 3 * c:3 * c + 1],
                                                 scale=1.0)
                        else:
                            nc.vector.tensor_scalar(dst, pc,
                                                    pt[0:csz, 3 * c:3 * c + 1],
                                                    None, OP.add)
                        # square on gpsimd, feeding the variance stats matmul
                        sqt = small.tile([csz, TW], b16, name=f"sq{t}_{c}",
                                         tag=f"sq{t}_{c}")
                        nc.gpsimd.tensor_mul(sqt, dst, dst)
                        sqs[(t, c)] = sqt
                # stats matmuls deferred so they don't stall PE on the
                # cast->square chain; by now early squares are done
                for t in range(NT):
                    tsl = slice(t * TW, (t + 1) * TW)
                    ps = psum_s.tile([128, TW], f32, tag="ps")
                    for c in range(4):
                        nc.tensor.matmul(ps, lhsT=ones_kk[0:CSZ[c], :],
                                         rhs=sqs[(t, c)],
                                         start=(c == 0), stop=(c == 3))
                    nc.scalar.activation(lnv[:, tsl], ps, AF.Ln,
                                         scale=1.0 / (HID * LN_EPS), bias=1.0)
                # rg = (var/eps + 1)^-0.5 = rstd * sqrt(eps); the eps^-0.5 and g
                # factors are folded into the v-scale param column
                rg = mid.tile([128, GW], b16, tag="rg")
                nc.scalar.activation(rg, lnv, AF.Exp, scale=-0.5)
                nxt = []
                for c in range(4):
                    csz = CSZ[c]
                    uu = mid.tile([csz, GW], b16, tag="uu")
                    nc.vector.tensor_mul(uu, ct[c], rg[0:csz, :])
                    vv = mid.tile([csz, GW], b16, tag="vv")
                    nc.vector.tensor_scalar(vv, uu, pt[0:csz, 3 * c + 1:3 * c + 2],
                                            pt[0:csz, 3 * c + 2:3 * c + 3],
                                            OP.mult, OP.add)
                    ee = mid.tile([csz, GW], b16, tag="ee")
                    nc.scalar.activation(ee, vv, AF.Exp)
                    # elu(v) = max(v, min(e^v, 1) - 1)  (exp-first, overflow-safe)
                    tt = mid.tile([csz, GW], b16, tag="tt")
                    nc.vector.tensor_scalar(tt, ee, 1.0, 1.0, OP.min, OP.subtract)
                    xc = xpool.tile([csz + (1 if c == 3 else 0), GW], b16,
                                    name=f"xn{c}", tag=f"xn{c}")
                    if c == 3 and l == NLAYERS - 1:
                        # row 16 doubles as the ones row feeding the head's
                        # bias matmul chunk; partition-16 writes are illegal
                        # so fill the whole tile first.
                        nc.gpsimd.memset(xc, 1.0)
                    nc.vector.tensor_max(xc[0:csz, :], vv, tt)
                    nxt.append(xc)
                cur = nxt
            return cur

        def value_head(nk, ws, g, x4):
            for t in range(NT):
                tsl = slice(t * TW, (t + 1) * TW)
                pv = psum_c.tile([1, TW], f32, tag="pc")
                for ki, kn in enumerate(KSZ_F):
                    nc.tensor.matmul(pv, lhsT=ws[("f", ki)], rhs=x4[ki][0:kn, tsl],
                                     start=(ki == 0), stop=(ki == 3))
                vtmp = small.tile([1, TW], f32, tag="vtmp")
                nc.scalar.copy(vtmp, pv)
                nc.sync.dma_start(out=vd[nk][g * NT + t:g * NT + t + 1, :], in_=vtmp)

        # ---- critic + target nets ----
        for nk in ("c", "t"):
            ws = load_weights(nk)
            for g in range(NG):
                x4 = mlp_group(nk, ws, g)
                value_head(nk, ws, g, x4)

        # ---- GAE / reality weight / value loss ----
        import concourse.bass as bass_mod
        v_sb = persist.tile([MC, J], f32, tag="v_sb")
        vt_sb = persist.tile([MC, J], f32, tag="vt_sb")
        for nk, dst in (("c", v_sb), ("t", vt_sb)):
            src = bass_mod.AP(tensor=vd[nk][:, :].tensor, offset=0,
                              ap=[[J, MC], [1, J]])
            nc.sync.dma_start(out=dst, in_=src)

        gae = persist
        q = gae.tile([MC, H], f32, tag="q")
        nc.vector.tensor_scalar(q, term[:, 0:H], -GAMMA, GAMMA, OP.mult, OP.add)
        s1 = gae.tile([MC, H], f32, tag="s1")
        nc.vector.tensor_mul(s1, q, vt_sb[:, 0:H])
        s2 = gae.tile([MC, H], f32, tag="s2")
        nc.vector.tensor_sub(s2, rew[:, 0:H], vt_sb[:, 1:J])
        adv = gae.tile([MC, H], f32, tag="adv")
        nc.vector.tensor_add(adv, s1, s2)
        cl = gae.tile([MC, H], f32, tag="cl")
        nc.vector.tensor_scalar(cl, q, LAMBDA, None, OP.mult)
        agae = gae.tile([MC, H], f32, tag="agae")
        nc.vector.tensor_tensor_scan(agae, cl, adv, 0.0, OP.mult, OP.add)
        vtar = gae.tile([MC, H], f32, tag="vtar")
        nc.vector.tensor_add(vtar, agae, vt_sb[:, 1:J])
        delta = gae.tile([MC, H], f32, tag="delta")
        nc.vector.tensor_sub(delta, vtar, v_sb[:, 1:J])

        lg = gae.tile([MC, H], f32, tag="lg")
        nc.scalar.activation(lg, term[:, 1:J], AF.Ln, scale=-1.0, bias=1.0)
        S = gae.tile([MC, 1], f32, tag="S")
        nc.vector.tensor_reduce(S, lg, axis=mybir.AxisListType.X, op=OP.add)
        ones_mc = gae.tile([MC, H], f32, tag="ones_mc")
        nc.vector.memset(ones_mc, 1.0)
        cinc = gae.tile([MC, H], f32, tag="cinc")
        nc.vector.tensor_tensor_scan(cinc, ones_mc, lg, 0.0, OP.mult, OP.add)
        cx = gae.tile([MC, H], f32, tag="cx")
        nc.vector.tensor_sub(cx, cinc, lg)
        rwf = gae.tile([MC, J], f32, tag="rwf")
        nc.vector.memset(rwf[:, 0:1], 0.0)
        nc.scalar.activation(rwf[:, 1:J], cx, AF.Exp, scale=-1.0, bias=S)
        gfull = gae.tile([MC, J], f32, tag="gfull")
        nc.vector.memset(gfull[:, 0:1], 0.0)
        nc.vector.tensor_mul(gfull[:, 1:J], agae, rwf[:, 1:J])

        d2 = gae.tile([MC, H], f32, tag="d2")
        nc.vector.tensor_mul(d2, delta, delta)
        val_acc = gae.tile([MC, 1], f32, tag="val_acc")
        scrap64 = gae.tile([MC, H], f32, tag="scrap64")
        nc.vector.scalar_tensor_tensor(scrap64, d2, 1.0, rwf[:, 1:J],
                                       OP.mult, OP.mult, accum_out=val_acc)
        nc.sync.dma_start(out=out_e[0:MC, 0:1], in_=val_acc)

        gdst = bass_mod.AP(tensor=gbuf[:].tensor, offset=0, ap=[[J, MC], [1, J]])
        rwdst = bass_mod.AP(tensor=rwbuf[:].tensor, offset=0, ap=[[J, MC], [1, J]])
        nc.sync.dma_start(out=gdst, in_=gfull)
        nc.sync.dma_start(out=rwdst, in_=rwf)

        # ---- actor net + policy head ----
        ws = load_weights("a")
        for g in range(NG):
            x4 = mlp_group("a", ws, g)
            pz = psum_s.tile([128, TW], f32, tag="ps")
            for b in range(4):
                bsl = slice(b * TW, (b + 1) * TW)
                for ki, kn in enumerate(KSZ_F):
                    nc.tensor.matmul(pz[32 * b:32 * b + 32, :], lhsT=ws[("f", ki)],
                                     rhs=x4[ki][0:kn, bsl],
                                     start=(ki == 0), stop=(ki == 3),
                                     tile_position=(0, 32 * b))
            a_sb = headp.tile([128, TW], b16, tag="a_sb")
            nc.sync.dma_start(out=a_sb, in_=act_e[g])
            E = headp.tile([128, TW], b16, tag="E")
            nc.scalar.activation(E, pz, AF.Exp)
            Ez = headp.tile([128, TW], b16, tag="Ez")
            nc.vector.tensor_mul(Ez, pz, E)
            az = headp.tile([128, TW], b16, tag="az")
            nc.vector.tensor_mul(az, pz, a_sb)
            sums = {}
            for qn, src in (("S", E), ("W", Ez), ("U", az), ("B", a_sb)):
                p = psum_s.tile([128, TW], f32, tag="ps")
                nc.tensor.matmul(p, lhsT=bdiag, rhs=src, start=True, stop=True)
                sums[qn] = p
            lnS = headp.tile([128, TW], f32, tag="lnS")
            nc.scalar.activation(lnS, sums["S"], AF.Ln)
            Sinv = headp.tile([128, TW], b16, tag="Sinv")
            nc.scalar.activation(Sinv, lnS, AF.Exp, scale=-1.0)
            t1 = headp.tile([128, TW], b16, tag="t1")
            nc.vector.tensor_mul(t1, sums["B"], lnS)
            alp = headp.tile([128, TW], b16, tag="alp")
            nc.vector.tensor_sub(alp, sums["U"], t1)
            t2 = headp.tile([128, TW], b16, tag="t2")
            nc.vector.tensor_mul(t2, sums["W"], Sinv)
            ent = headp.tile([128, TW], b16, tag="ent")
            nc.vector.tensor_sub(ent, lnS, t2)

            gstk = headp.tile([128, TW], f32, tag="gstk")
            gsrc = bass_mod.AP(tensor=gbuf[:].tensor, offset=g * GW,
                               ap=[[TW, 4], [0, 32], [1, TW]])
            nc.gpsimd.dma_start(out=gstk, in_=gsrc)
            rstk = headp.tile([128, TW], f32, tag="rstk")
            rsrc = bass_mod.AP(tensor=rwbuf[:].tensor, offset=g * GW,
                               ap=[[TW, 4], [0, 32], [1, TW]])
            nc.gpsimd.dma_start(out=rstk, in_=rsrc)
            scrap = headp.tile([128, TW], b16, tag="scrap")
            nc.vector.scalar_tensor_tensor(scrap, alp, 1.0, gstk, OP.mult, OP.mult,
                                           accum_out=pol_cols[:, g:g + 1])
            scrap2 = headp.tile([128, TW], b16, tag="scrap2")
            nc.vector.scalar_tensor_tensor(scrap2, ent, 1.0, rstk, OP.mult, OP.mult,
                                           accum_out=ent_cols[:, g:g + 1])

        pol_sum = persist.tile([128, 1], f32, tag="pol_sum")
        nc.vector.tensor_reduce(pol_sum, pol_cols, axis=mybir.AxisListType.X, op=OP.add)
        ent_sum = persist.tile([128, 1], f32, tag="ent_sum")
        nc.vector.tensor_reduce(ent_sum, ent_cols, axis=mybir.AxisListType.X, op=OP.add)
        nc.sync.dma_start(out=out_e[:, 1:2], in_=pol_sum)
        nc.sync.dma_start(out=out_e[:, 2:3], in_=ent_sum)

    import concourse.bacc as bacc_mod
    import concourse.hw_specs as hw_specs
    orig_tables = hw_specs.get_activation_tables
    keep = "natural_log_exp_and_others"
    mine = {AF.Identity, AF.Copy, AF.Exp, AF.Ln, AF.Square}

    def patched_tables(arch):
        out = {}
        for name, s in orig_tables(arch).items():
            out[name] = s if name == keep else (s - mine)
        return out

    bacc_mod.get_activation_tables = patched_tables
    try:
        nc.compile()
    finally:
        bacc_mod.get_activation_tables = orig_tables
    return nc


_NC_CACHE = {}


def kernel(features, rewards_mean, terminals_mean, actions,
           actor_params, critic_params, target_params, _want_timing=False):
    from concourse.bass_utils import run_bass_kernel_spmd

    in_maps = prep_all(features, rewards_mean, terminals_mean, actions,
                       actor_params, critic_params, target_params)
    if "nc" not in _NC_CACHE:
        _NC_CACHE["nc"] = build_graph()
    nc = _NC_CACHE["nc"]
    res = run_bass_kernel_spmd(nc, in_maps, core_ids=list(range(NCORES)),
                               trace=_want_timing)
    outs = [np.asarray(r["out"], np.float64) for r in res.results]
    val = sum(o[0:MC, 0].sum() for o in outs)
    pol = sum(o[:, 1].sum() for o in outs) / 32.0
    ent = sum(o[:, 2].sum() for o in outs) / 32.0
    denom = float(H * M)
    loss_value = 0.5 * val / denom
    loss_policy = -pol / denom
    policy_entropy = ent / denom
    loss_actor = loss_policy - TEMPERATURE * policy_entropy
    out = np.array([loss_actor, loss_value], np.float32)
    if _want_timing:
        return out, res
    return out


# revision 21
# speedup vs baseline: 1.2167x; 1.2167x over previous
"""Trainium2 Bass kernel for nn_ActorCritic loss (8-core SPMD, data-parallel over batch).

Strategy:
- Shard batch M=512 over 8 cores (64 each). MLP params replicated.
- Host prep: features pre-transposed to feature-major [F, T] per core with token
  column t = m*J + u, u = J-1-j (time-reversed so the GAE recursion becomes a
  forward hardware scan). LayerNorm mean-subtraction is folded into the weights
  (W - rowmean, b - mean); LN gain g is folded into the rstd computation via
  exp(-0.5*ln(var+eps) + ln g).
- On chip per core: 3 MLPs (critic/target/actor) in feature-major layout, bf16
  matmuls with fp32 PSUM. Per-token variance via ones-matrix matmul broadcast.
  ELU via exp/min/max with scalar_tensor_tensor fusion. GAE via
  tensor_tensor_scan. Reality weight via suffix-cumsum trick (no reversal).
  Actor head: stacked [4x32, 512] logits tiles, block-diagonal ones matmul for
  per-token reductions over A.
- Each core emits partial loss sums; host combines to the final [2] output.
"""
import os
import sys
import numpy as np

for _p in ("/opt/trn_rl_repo",):
    if _p not in sys.path and os.path.isdir(_p):
        sys.path.insert(0, _p)

import ml_dtypes  # noqa: E402

bf16 = ml_dtypes.bfloat16

GAMMA = 0.999
LAMBDA = 0.95
TEMPERATURE = 0.001
J, M, F, A = 128, 512, 256, 32
H = J - 1
HID, NLAYERS = 400, 4
LN_EPS = 1e-3
NCORES = 8
MC = M // NCORES          # 64 batch elements per core
T = MC * J                # 8192 tokens per core
NG = 4                    # token groups
GW = T // NG              # 2048 cols per group
NT = 4                    # 512-col tiles per group
TW = GW // NT             # 512

CSZ = [128, 128, 128, 16]         # dout chunks of HID=400
COFS = [0, 128, 256, 384]
KSZ_L0 = [128, 128]               # din chunks of F=256
KSZ_H = [128, 128, 128, 16]       # din chunks of HID=400
KSZ_F = [128, 128, 128, 17]       # din chunks of HID+1 (bias row)

_NETS = [("c", 1), ("t", 1), ("a", 32)]


# ----------------------------------------------------------------------------
# host-side prep
# ----------------------------------------------------------------------------

def _prep_net(params):
    hidden, (Wf, bf_) = params
    pre = []
    for (W, b, g, beta) in hidden:
        W = np.asarray(W, np.float32)
        b = np.asarray(b, np.float32)
        g = np.asarray(g, np.float32)
        beta = np.asarray(beta, np.float32)
        Wc = W - W.mean(axis=1, keepdims=True)
        bc = b - b.mean()
        if np.any(g <= 0):
            raise ValueError("LN gain fold requires g > 0")
        pre.append((Wc.astype(bf16), bc, np.log(g), beta))
    Wf = np.asarray(Wf, np.float32)
    bf_ = np.asarray(bf_, np.float32)
    Wf_aug = np.concatenate([Wf, bf_[None, :]], axis=0).astype(bf16)
    return pre, Wf_aug


def _param_tile(pre_l):
    """[128, 12] fp32: col 3c+0 = centered bias, 3c+1 = ln g, 3c+2 = beta for chunk c."""
    _, bc, lng, beta = pre_l
    out = np.zeros((128, 12), np.float32)
    for c in range(4):
        n = CSZ[c]
        sl = slice(COFS[c], COFS[c] + n)
        out[:n, 3 * c + 0] = bc[sl]
        out[:n, 3 * c + 1] = np.exp(lng[sl]) * LN_EPS ** -0.5
        out[:n, 3 * c + 2] = beta[sl]
    return out


def prep_all(features, rewards_mean, terminals_mean, actions,
             actor_params, critic_params, target_params):
    features = np.asarray(features, np.float32)
    rewards_mean = np.asarray(rewards_mean, np.float32)
    terminals_mean = np.asarray(terminals_mean, np.float32)
    actions = np.asarray(actions, np.float32)

    nets = {}
    for key, params in (("c", critic_params), ("t", target_params), ("a", actor_params)):
        nets[key] = _prep_net(params)

    shared = {"bdiag": np.kron(np.eye(4, dtype=np.float32),
                               np.ones((32, 32), np.float32)).astype(bf16)}
    for nk, (pre, Wf_aug) in nets.items():
        shared[f"wf_{nk}"] = np.ascontiguousarray(Wf_aug)
        for l, p in enumerate(pre):
            shared[f"w_{nk}_{l}"] = np.ascontiguousarray(p[0])
            shared[f"p_{nk}_{l}"] = _param_tile(p)

    in_maps = []
    for core in range(NCORES):
        msl = slice(core * MC, (core + 1) * MC)
        d = dict(shared)
        x0 = features[::-1, msl, :].transpose(2, 1, 0).reshape(F, T)
        d["x0"] = np.ascontiguousarray(x0.astype(bf16))
        d["rew"] = np.ascontiguousarray(rewards_mean[::-1, msl].T)
        d["term"] = np.ascontiguousarray(terminals_mean[::-1, msl].T)
        act_u = np.zeros((J, MC, A), np.float32)
        act_u[1:] = actions[::-1, msl, :]
        act_T = act_u.transpose(2, 1, 0).reshape(A, T)          # [32, 8192]
        act_stk = act_T.reshape(A, NG, NT, TW).transpose(1, 2, 0, 3).reshape(NG, 128, TW)
        d["act_stk"] = np.ascontiguousarray(act_stk.astype(bf16))
        in_maps.append(d)
    return in_maps


# ----------------------------------------------------------------------------
# bass graph
# ----------------------------------------------------------------------------

def build_graph():
    import contextlib
    import concourse.bass as bass
    import concourse.tile as tile
    from concourse import bacc, mybir

    f32 = mybir.dt.float32
    b16 = mybir.dt.bfloat16
    AF = mybir.ActivationFunctionType
    OP = mybir.AluOpType

    nc = bacc.Bacc()

    x0_e = nc.declare_dram_parameter("x0", [F, T], b16, isOutput=False)
    rew_e = nc.declare_dram_parameter("rew", [MC, J], f32, isOutput=False)
    term_e = nc.declare_dram_parameter("term", [MC, J], f32, isOutput=False)
    act_e = nc.declare_dram_parameter("act_stk", [NG, 128, TW], b16, isOutput=False)
    bdiag_e = nc.declare_dram_parameter("bdiag", [128, 128], b16, isOutput=False)
    w_e, p_e, wf_e = {}, {}, {}
    for nk, dout in _NETS:
        wf_e[nk] = nc.declare_dram_parameter(f"wf_{nk}", [HID + 1, dout], b16, isOutput=False)
        for l in range(NLAYERS):
            din = F if l == 0 else HID
            w_e[(nk, l)] = nc.declare_dram_parameter(f"w_{nk}_{l}", [din, HID], b16, isOutput=False)
            p_e[(nk, l)] = nc.declare_dram_parameter(f"p_{nk}_{l}", [128, 12], f32, isOutput=False)
    out_e = nc.declare_dram_parameter("out", [128, 8], f32, isOutput=True)

    vd = {nk: nc.dram_tensor(f"vd_{nk}", [NG * NT, TW], f32) for nk in ("c", "t")}
    gbuf = nc.dram_tensor("gbuf", [T], f32)
    rwbuf = nc.dram_tensor("rwbuf", [T], f32)

    with tile.TileContext(nc) as tc, contextlib.ExitStack() as ctx:
        persist = ctx.enter_context(tc.tile_pool(name="persist", bufs=1))
        wpool = ctx.enter_context(tc.tile_pool(name="wpool", bufs=1))
        xpool = ctx.enter_context(tc.tile_pool(name="xpool", bufs=2))
        x0pool = ctx.enter_context(tc.tile_pool(name="x0pool", bufs=1))
        mid = ctx.enter_context(tc.tile_pool(name="mid", bufs=2))
        ctp = ctx.enter_context(tc.tile_pool(name="ctp", bufs=2))
        small = ctx.enter_context(tc.tile_pool(name="small", bufs=2))
        headp = ctx.enter_context(tc.tile_pool(name="headp", bufs=1))
        psum_c = ctx.enter_context(tc.tile_pool(name="psum_c", bufs=4, space="PSUM"))
        psum_s = ctx.enter_context(tc.tile_pool(name="psum_s", bufs=4, space="PSUM"))

        # --- constants ---
        ones_kk = persist.tile([128, 128], b16, tag="ones_kk")
        nc.vector.memset(ones_kk, 1.0)
        bdiag = persist.tile([128, 128], b16, tag="bdiag")
        nc.sync.dma_start(out=bdiag, in_=bdiag_e[:, :])
        rew = persist.tile([MC, J], f32, tag="rew")
        nc.sync.dma_start(out=rew, in_=rew_e[:, :])
        term = persist.tile([MC, J], f32, tag="term")
        nc.sync.dma_start(out=term, in_=term_e[:, :])
        pol_cols = persist.tile([128, NG], f32, tag="pol_cols")
        ent_cols = persist.tile([128, NG], f32, tag="ent_cols")

        ptiles = {}
        for nk, _ in _NETS:
            for l in range(NLAYERS):
                pt = persist.tile([128, 12], f32, tag=f"p_{nk}_{l}")
                nc.sync.dma_start(out=pt, in_=p_e[(nk, l)][:, :])
                pt2 = persist.tile([128, 12], f32, name=f"p2_{nk}_{l}",
                                   tag=f"p2_{nk}_{l}")
                nc.scalar.copy(pt2, pt)
                ptiles[(nk, l)] = pt2

        def load_weights(nk):
            """DMA one net's weights into SBUF chunk tiles."""
            ws = {}
            for l in range(NLAYERS):
                ksz = KSZ_L0 if l == 0 else KSZ_H
                kofs = 0
                for ki, kn in enumerate(ksz):
                    wt = wpool.tile([kn, HID], b16, tag=f"w_l{l}_k{ki}")
                    nc.sync.dma_start(out=wt, in_=w_e[(nk, l)][kofs:kofs + kn, :])
                    ws[(l, ki)] = wt
                    kofs += kn
            dout = dict(_NETS)[nk]
            kofs = 0
            for ki, kn in enumerate(KSZ_F):
                wt = wpool.tile([kn, dout], b16, tag=f"wf_k{ki}")
                nc.sync.dma_start(out=wt, in_=wf_e[nk][kofs:kofs + kn, :])
                ws[("f", ki)] = wt
                kofs += kn
            return ws

        def mlp_group(nk, ws, g):
            """Run 4 hidden layers for token group g. Returns X4 chunk tiles
            ([csz, GW], chunk 3 is [17, GW] with a ones row at 16)."""
            gofs = g * GW
            xin = []
            for ki, kn in enumerate(KSZ_L0):
                xt = x0pool.tile([kn, GW], b16, name=f"x0_{ki}", tag=f"x0_{ki}")
                nc.sync.dma_start(out=xt, in_=x0_e[ki * 128:ki * 128 + kn, gofs:gofs + GW])
                xin.append(xt)

            cur = xin
            for l in range(NLAYERS):
                ksz = KSZ_L0 if l == 0 else KSZ_H
                pt = ptiles[(nk, l)]
                ct = [ctp.tile([CSZ[c], GW], b16, name=f"ct{c}", tag=f"ct{c}")
                      for c in range(4)]
                lnv = mid.tile([128, GW], b16, tag="lnv")
                sqs = {}
                for t in range(NT):
                    tsl = slice(t * TW, (t + 1) * TW)
                    for c in range(4):
                        csz = CSZ[c]
                        pc = psum_c.tile([csz, TW], f32, tag="pc")
                        for ki, kn in enumerate(ksz):
                            nc.tensor.matmul(
                                pc, lhsT=ws[(l, ki)][:, COFS[c]:COFS[c] + csz],
                                rhs=cur[ki][0:kn, tsl],
                                start=(ki == 0), stop=(ki == len(ksz) - 1))
                        # PSUM -> SBUF cast with centered-bias add; split ACT/DVE
                        dst = ct[c][:, tsl]
                        if c < 2:
                            nc.scalar.activation(dst, pc, AF.Identity,
                                                 bias=pt[0:csz, 3 * c:3 * c + 1],
                                                 scale=1.0)
                        else:
                            nc.vector.tensor_scalar(dst, pc,
                                                    pt[0:csz, 3 * c:3 * c + 1],
                                                    None, OP.add)
                        # square on gpsimd, feeding the variance stats matmul
                        sqt = small.tile([csz, TW], b16, name=f"sq{t}_{c}",
                                         tag=f"sq{t}_{c}")
                        nc.gpsimd.tensor_mul(sqt, dst, dst)
                        sqs[(t, c)] = sqt
                # stats matmuls deferred so they don't stall PE on the
                # cast->square chain; by now early squares are done
                for t in range(NT):
                    tsl = slice(t * TW, (t + 1) * TW)
                    ps = psum_s.tile([128, TW], f32, tag="ps")
                    for c in range(4):
                        nc.tensor.matmul(ps, lhsT=ones_kk[0:CSZ[c], :],
                                         rhs=sqs[(t, c)],
                                         start=(c == 0), stop=(c == 3))
                    nc.scalar.activation(lnv[:, tsl], ps, AF.Ln,
                                         scale=1.0 / (HID * LN_EPS), bias=1.0)
                # rg = (var/eps + 1)^-0.5 = rstd * sqrt(eps); the eps^-0.5 and g
                # factors are folded into the v-scale param column
                rg = mid.tile([128, GW], b16, tag="rg")
                nc.scalar.activation(rg, lnv, AF.Exp, scale=-0.5)
                nxt = []
                for c in range(4):
                    csz = CSZ[c]
                    uu = mid.tile([csz, GW], b16, tag="uu")
                    nc.vector.tensor_mul(uu, ct[c], rg[0:csz, :])
                    vv = mid.tile([csz, GW], b16, tag="vv")
                    nc.vector.tensor_scalar(vv, uu, pt[0:csz, 3 * c + 1:3 * c + 2],
                                            pt[0:csz, 3 * c + 2:3 * c + 3],
                                            OP.mult, OP.add)
                    ee = mid.tile([csz, GW], b16, tag="ee")
                    nc.scalar.activation(ee, vv, AF.Exp)
                    # elu(v) = max(v, min(e^v, 1) - 1)  (exp-first, overflow-safe)
                    tt = mid.tile([csz, GW], b16, tag="tt")
                    nc.vector.tensor_scalar(tt, ee, 1.0, 1.0, OP.min, OP.subtract)
                    xc = xpool.tile([csz + (1 if c == 3 else 0), GW], b16,
                                    name=f"xn{c}", tag=f"xn{c}")
                    if c == 3 and l == NLAYERS - 1:
                        # row 16 doubles as the ones row feeding the head's
                        # bias matmul chunk; partition-16 writes are illegal
                        # so fill the whole tile first.
                        nc.gpsimd.memset(xc, 1.0)
                    nc.vector.tensor_max(xc[0:csz, :], vv, tt)
                    nxt.append(xc)
                cur = nxt
            return cur

        def value_head(nk, ws, g, x4):
            for t in range(NT):
                tsl = slice(t * TW, (t + 1) * TW)
                pv = psum_c.tile([1, TW], f32, tag="pc")
                for ki, kn in enumerate(KSZ_F):
                    nc.tensor.matmul(pv, lhsT=ws[("f", ki)], rhs=x4[ki][0:kn, tsl],
                                     start=(ki == 0), stop=(ki == 3))
                vtmp = small.tile([1, TW], f32, tag="vtmp")
                nc.scalar.copy(vtmp, pv)
                nc.sync.dma_start(out=vd[nk][g * NT + t:g * NT + t + 1, :], in_=vtmp)

        # ---- critic + target nets ----
        for nk in ("c", "t"):
            ws = load_weights(nk)
            for g in range(NG):
                x4 = mlp_group(nk, ws, g)
                value_head(nk, ws, g, x4)

        # ---- GAE / reality weight / value loss ----
        import concourse.bass as bass_mod
        v_sb = persist.tile([MC, J], f32, tag="v_sb")
        vt_sb = persist.tile([MC, J], f32, tag="vt_sb")
        for nk, dst in (("c", v_sb), ("t", vt_sb)):
            src = bass_mod.AP(tensor=vd[nk][:, :].tensor, offset=0,
                              ap=[[J, MC], [1, J]])
            nc.sync.dma_start(out=dst, in_=src)

        gae = persist
        q = gae.tile([MC, H], f32, tag="q")
        nc.vector.tensor_scalar(q, term[:, 0:H], -GAMMA, GAMMA, OP.mult, OP.add)
        s1 = gae.tile([MC, H], f32, tag="s1")
        nc.vector.tensor_mul(s1, q, vt_sb[:, 0:H])
        s2 = gae.tile([MC, H], f32, tag="s2")
        nc.vector.tensor_sub(s2, rew[:, 0:H], vt_sb[:, 1:J])
        adv = gae.tile([MC, H], f32, tag="adv")
        nc.vector.tensor_add(adv, s1, s2)
        cl = gae.tile([MC, H], f32, tag="cl")
        nc.vector.tensor_scalar(cl, q, LAMBDA, None, OP.mult)
        agae = gae.tile([MC, H], f32, tag="agae")
        nc.vector.tensor_tensor_scan(agae, cl, adv, 0.0, OP.mult, OP.add)
        vtar = gae.tile([MC, H], f32, tag="vtar")
        nc.vector.tensor_add(vtar, agae, vt_sb[:, 1:J])
        delta = gae.tile([MC, H], f32, tag="delta")
        nc.vector.tensor_sub(delta, vtar, v_sb[:, 1:J])

        lg = gae.tile([MC, H], f32, tag="lg")
        nc.scalar.activation(lg, term[:, 1:J], AF.Ln, scale=-1.0, bias=1.0)
        S = gae.tile([MC, 1], f32, tag="S")
        nc.vector.tensor_reduce(S, lg, axis=mybir.AxisListType.X, op=OP.add)
        ones_mc = gae.tile([MC, H], f32, tag="ones_mc")
        nc.vector.memset(ones_mc, 1.0)
        cinc = gae.tile([MC, H], f32, tag="cinc")
        nc.vector.tensor_tensor_scan(cinc, ones_mc, lg, 0.0, OP.mult, OP.add)
        cx = gae.tile([MC, H], f32, tag="cx")
        nc.vector.tensor_sub(cx, cinc, lg)
        rwf = gae.tile([MC, J], f32, tag="rwf")
        nc.vector.memset(rwf[:, 0:1], 0.0)
        nc.scalar.activation(rwf[:, 1:J], cx, AF.Exp, scale=-1.0, bias=S)
        gfull = gae.tile([MC, J], f32, tag="gfull")
        nc.vector.memset(gfull[:, 0:1], 0.0)
        nc.vector.tensor_mul(gfull[:, 1:J], agae, rwf[:, 1:J])

        d2 = gae.tile([MC, H], f32, tag="d2")
        nc.vector.tensor_mul(d2, delta, delta)
        val_acc = gae.tile([MC, 1], f32, tag="val_acc")
        scrap64 = gae.tile([MC, H], f32, tag="scrap64")
        nc.vector.scalar_tensor_tensor(scrap64, d2, 1.0, rwf[:, 1:J],
                                       OP.mult, OP.mult, accum_out=val_acc)
        nc.sync.dma_start(out=out_e[0:MC, 0:1], in_=val_acc)

        gdst = bass_mod.AP(tensor=gbuf[:].tensor, offset=0, ap=[[J, MC], [1, J]])
        rwdst = bass_mod.AP(tensor=rwbuf[:].tensor, offset=0, ap=[[J, MC], [1, J]])
        nc.sync.dma_start(out=gdst, in_=gfull)
        nc.sync.dma_start(out=rwdst, in_=rwf)

        # ---- actor net + policy head ----
        ws = load_weights("a")
        for g in range(NG):
            x4 = mlp_group("a", ws, g)
            pz = psum_s.tile([128, TW], f32, tag="ps")
            for b in range(4):
                bsl = slice(b * TW, (b + 1) * TW)
                for ki, kn in enumerate(KSZ_F):
                    nc.tensor.matmul(pz[32 * b:32 * b + 32, :], lhsT=ws[("f", ki)],
                                     rhs=x4[ki][0:kn, bsl],
                                     start=(ki == 0), stop=(ki == 3),
                                     tile_position=(0, 32 * b))
            a_sb = headp.tile([128, TW], b16, tag="a_sb")
            nc.sync.dma_start(out=a_sb, in_=act_e[g])
            E = headp.tile([128, TW], b16, tag="E")
            nc.scalar.activation(E, pz, AF.Exp)
            Ez = headp.tile([128, TW], b16, tag="Ez")
            nc.vector.tensor_mul(Ez, pz, E)
            az = headp.tile([128, TW], b16, tag="az")
            nc.vector.tensor_mul(az, pz, a_sb)
            sums = {}
            for qn, src in (("S", E), ("W", Ez), ("U", az), ("B", a_sb)):
                p = psum_s.tile([128, TW], f32, tag="ps")
                nc.tensor.matmul(p, lhsT=bdiag, rhs=src, start=True, stop=True)
                sums[qn] = p
            lnS = headp.tile([128, TW], f32, tag="lnS")
            nc.scalar.activation(lnS, sums["S"], AF.Ln)
            Sinv = headp.tile([128, TW], b16, tag="Sinv")
            nc.scalar.activation(Sinv, lnS, AF.Exp, scale=-1.0)
            t1 = headp.tile([128, TW], b16, tag="t1")
            nc.vector.tensor_mul(t1, sums["B"], lnS)
            alp = headp.tile([128, TW], b16, tag="alp")
            nc.vector.tensor_sub(alp, sums["U"], t1)
            t2 = headp.tile([128, TW], b16, tag="t2")
            nc.vector.tensor_mul(t2, sums["W"], Sinv)
            ent = headp.tile([128, TW], b16, tag="ent")
            nc.vector.tensor_sub(ent, lnS, t2)

            gstk = headp.tile([128, TW], f32, tag="gstk")
            gsrc = bass_mod.AP(tensor=gbuf[:].tensor, offset=g * GW,
                               ap=[[TW, 4], [0, 32], [1, TW]])
            nc.gpsimd.dma_start(out=gstk, in_=gsrc)
            rstk = headp.tile([128, TW], f32, tag="rstk")
            rsrc = bass_mod.AP(tensor=rwbuf[:].tensor, offset=g * GW,
                               ap=[[TW, 4], [0, 32], [1, TW]])
            nc.gpsimd.dma_start(out=rstk, in_=rsrc)
            scrap = headp.tile([128, TW], b16, tag="scrap")
            nc.vector.scalar_tensor_tensor(scrap, alp, 1.0, gstk, OP.mult, OP.mult,
                                           accum_out=pol_cols[:, g:g + 1])
            scrap2 = headp.tile([128, TW], b16, tag="scrap2")
            nc.vector.scalar_tensor_tensor(scrap2, ent, 1.0, rstk, OP.mult, OP.mult,
                                           accum_out=ent_cols[:, g:g + 1])

        pol_sum = persist.tile([128, 1], f32, tag="pol_sum")
        nc.vector.tensor_reduce(pol_sum, pol_cols, axis=mybir.AxisListType.X, op=OP.add)
        ent_sum = persist.tile([128, 1], f32, tag="ent_sum")
        nc.vector.tensor_reduce(ent_sum, ent_cols, axis=mybir.AxisListType.X, op=OP.add)
        nc.sync.dma_start(out=out_e[:, 1:2], in_=pol_sum)
        nc.sync.dma_start(out=out_e[:, 2:3], in_=ent_sum)

    import concourse.bacc as bacc_mod
    import concourse.hw_specs as hw_specs
    orig_tables = hw_specs.get_activation_tables
    keep = "natural_log_exp_and_others"
    mine = {AF.Identity, AF.Copy, AF.Exp, AF.Ln, AF.Square}

    def patched_tables(arch):
        out = {}
        for name, s in orig_tables(arch).items():
            out[name] = s if name == keep else (s - mine)
        return out

    bacc_mod.get_activation_tables = patched_tables
    try:
        nc.compile()
    finally:
        bacc_mod.get_activation_tables = orig_tables
    return nc


_NC_CACHE = {}


def kernel(features, rewards_mean, terminals_mean, actions,
           actor_params, critic_params, target_params, _want_timing=False):
    from concourse.bass_utils import run_bass_kernel_spmd

    in_maps = prep_all(features, rewards_mean, terminals_mean, actions,
                       actor_params, critic_params, target_params)
    if "nc" not in _NC_CACHE:
        _NC_CACHE["nc"] = build_graph()
    nc = _NC_CACHE["nc"]
    res = run_bass_kernel_spmd(nc, in_maps, core_ids=list(range(NCORES)),
                               trace=_want_timing)
    outs = [np.asarray(r["out"], np.float64) for r in res.results]
    val = sum(o[0:MC, 0].sum() for o in outs)
    pol = sum(o[:, 1].sum() for o in outs) / 32.0
    ent = sum(o[:, 2].sum() for o in outs) / 32.0
    denom = float(H * M)
    loss_value = 0.5 * val / denom
    loss_policy = -pol / denom
    policy_entropy = ent / denom
    loss_actor = loss_policy - TEMPERATURE * policy_entropy
    out = np.array([loss_actor, loss_value], np.float32)
    if _want_timing:
        return out, res
    return out


# revision 23
# speedup vs baseline: 1.2785x; 1.0508x over previous
"""Trainium2 Bass kernel for nn_ActorCritic loss (8-core SPMD, data-parallel over batch).

Strategy:
- Shard batch M=512 over 8 cores (64 each). MLP params replicated.
- Host prep: features pre-transposed to feature-major [F, T] per core with token
  column t = m*J + u, u = J-1-j (time-reversed so the GAE recursion becomes a
  forward hardware scan). LayerNorm mean-subtraction is folded into the weights
  (W - rowmean, b - mean); LN gain g is folded into the rstd computation via
  exp(-0.5*ln(var+eps) + ln g).
- On chip per core: 3 MLPs (critic/target/actor) in feature-major layout, bf16
  matmuls with fp32 PSUM. Per-token variance via ones-matrix matmul broadcast.
  ELU via exp/min/max with scalar_tensor_tensor fusion. GAE via
  tensor_tensor_scan. Reality weight via suffix-cumsum trick (no reversal).
  Actor head: stacked [4x32, 512] logits tiles, block-diagonal ones matmul for
  per-token reductions over A.
- Each core emits partial loss sums; host combines to the final [2] output.
"""
import os
import sys
import numpy as np

for _p in ("/opt/trn_rl_repo",):
    if _p not in sys.path and os.path.isdir(_p):
        sys.path.insert(0, _p)

import ml_dtypes  # noqa: E402

bf16 = ml_dtypes.bfloat16

GAMMA = 0.999
LAMBDA = 0.95
TEMPERATURE = 0.001
J, M, F, A = 128, 512, 256, 32
H = J - 1
HID, NLAYERS = 400, 4
LN_EPS = 1e-3
NCORES = 8
MC = M // NCORES          # 64 batch elements per core
T = MC * J                # 8192 tokens per core
NG = 4                    # token groups
GW = T // NG              # 2048 cols per group
NT = 4                    # 512-col tiles per group
TW = GW // NT             # 512

CSZ = [128, 128, 128, 16]         # dout chunks of HID=400
COFS = [0, 128, 256, 384]
KSZ_L0 = [128, 128]               # din chunks of F=256
KSZ_H = [128, 128, 128, 16]       # din chunks of HID=400
KSZ_F = [128, 128, 128, 17]       # din chunks of HID+1 (bias row)

_NETS = [("c", 1), ("t", 1), ("a", 32)]


# ----------------------------------------------------------------------------
# host-side prep
# ----------------------------------------------------------------------------

def _prep_net(params):
    hidden, (Wf, bf_) = params
    pre = []
    for (W, b, g, beta) in hidden:
        W = np.asarray(W, np.float32)
        b = np.asarray(b, np.float32)
        g = np.asarray(g, np.float32)
        beta = np.asarray(beta, np.float32)
        Wc = W - W.mean(axis=1, keepdims=True)
        bc = b - b.mean()
        if np.any(g <= 0):
            raise ValueError("LN gain fold requires g > 0")
        pre.append((Wc.astype(bf16), bc, np.log(g), beta))
    Wf = np.asarray(Wf, np.float32)
    bf_ = np.asarray(bf_, np.float32)
    Wf_aug = np.concatenate([Wf, bf_[None, :]], axis=0).astype(bf16)
    return pre, Wf_aug


def _param_tile(pre_l):
    """[128, 12] fp32: col 3c+0 = centered bias, 3c+1 = ln g, 3c+2 = beta for chunk c."""
    _, bc, lng, beta = pre_l
    out = np.zeros((128, 12), np.float32)
    for c in range(4):
        n = CSZ[c]
        sl = slice(COFS[c], COFS[c] + n)
        out[:n, 3 * c + 0] = bc[sl]
        out[:n, 3 * c + 1] = np.exp(lng[sl]) * LN_EPS ** -0.5
        out[:n, 3 * c + 2] = beta[sl]
    return out


def prep_all(features, rewards_mean, terminals_mean, actions,
             actor_params, critic_params, target_params):
    features = np.asarray(features, np.float32)
    rewards_mean = np.asarray(rewards_mean, np.float32)
    terminals_mean = np.asarray(terminals_mean, np.float32)
    actions = np.asarray(actions, np.float32)

    nets = {}
    for key, params in (("c", critic_params), ("t", target_params), ("a", actor_params)):
        nets[key] = _prep_net(params)

    shared = {"bdiag": np.kron(np.eye(4, dtype=np.float32),
                               np.ones((32, 32), np.float32)).astype(bf16)}
    for nk, (pre, Wf_aug) in nets.items():
        shared[f"wf_{nk}"] = np.ascontiguousarray(Wf_aug)
        for l, p in enumerate(pre):
            shared[f"w_{nk}_{l}"] = np.ascontiguousarray(p[0])
            shared[f"p_{nk}_{l}"] = _param_tile(p)

    in_maps = []
    for core in range(NCORES):
        msl = slice(core * MC, (core + 1) * MC)
        d = dict(shared)
        x0 = features[::-1, msl, :].transpose(2, 1, 0).reshape(F, T)
        d["x0"] = np.ascontiguousarray(x0.astype(bf16))
        d["rew"] = np.ascontiguousarray(rewards_mean[::-1, msl].T)
        d["term"] = np.ascontiguousarray(terminals_mean[::-1, msl].T)
        act_u = np.zeros((J, MC, A), np.float32)
        act_u[1:] = actions[::-1, msl, :]
        act_T = act_u.transpose(2, 1, 0).reshape(A, T)          # [32, 8192]
        act_stk = act_T.reshape(A, NG, NT, TW).transpose(1, 2, 0, 3).reshape(NG, 128, TW)
        d["act_stk"] = np.ascontiguousarray(act_stk.astype(bf16))
        in_maps.append(d)
    return in_maps


# ----------------------------------------------------------------------------
# bass graph
# ----------------------------------------------------------------------------

def build_graph():
    import contextlib
    import concourse.bass as bass
    import concourse.tile as tile
    from concourse import bacc, mybir

    f32 = mybir.dt.float32
    b16 = mybir.dt.bfloat16
    AF = mybir.ActivationFunctionType
    OP = mybir.AluOpType

    nc = bacc.Bacc()

    x0_e = nc.declare_dram_parameter("x0", [F, T], b16, isOutput=False)
    rew_e = nc.declare_dram_parameter("rew", [MC, J], f32, isOutput=False)
    term_e = nc.declare_dram_parameter("term", [MC, J], f32, isOutput=False)
    act_e = nc.declare_dram_parameter("act_stk", [NG, 128, TW], b16, isOutput=False)
    bdiag_e = nc.declare_dram_parameter("bdiag", [128, 128], b16, isOutput=False)
    w_e, p_e, wf_e = {}, {}, {}
    for nk, dout in _NETS:
        wf_e[nk] = nc.declare_dram_parameter(f"wf_{nk}", [HID + 1, dout], b16, isOutput=False)
        for l in range(NLAYERS):
            din = F if l == 0 else HID
            w_e[(nk, l)] = nc.declare_dram_parameter(f"w_{nk}_{l}", [din, HID], b16, isOutput=False)
            p_e[(nk, l)] = nc.declare_dram_parameter(f"p_{nk}_{l}", [128, 12], f32, isOutput=False)
    out_e = nc.declare_dram_parameter("out", [128, 8], f32, isOutput=True)

    vd = {nk: nc.dram_tensor(f"vd_{nk}", [NG * NT, TW], f32) for nk in ("c", "t")}
    gbuf = nc.dram_tensor("gbuf", [T], f32)
    rwbuf = nc.dram_tensor("rwbuf", [T], f32)

    with tile.TileContext(nc) as tc, contextlib.ExitStack() as ctx:
        persist = ctx.enter_context(tc.tile_pool(name="persist", bufs=1))
        wpool = ctx.enter_context(tc.tile_pool(name="wpool", bufs=1))
        xpool = ctx.enter_context(tc.tile_pool(name="xpool", bufs=2))
        x0pool = ctx.enter_context(tc.tile_pool(name="x0pool", bufs=1))
        mid = ctx.enter_context(tc.tile_pool(name="mid", bufs=1))
        ctp = ctx.enter_context(tc.tile_pool(name="ctp", bufs=1))
        small = ctx.enter_context(tc.tile_pool(name="small", bufs=2))
        headp = ctx.enter_context(tc.tile_pool(name="headp", bufs=1))
        psum_c = ctx.enter_context(tc.tile_pool(name="psum_c", bufs=4, space="PSUM"))
        psum_s = ctx.enter_context(tc.tile_pool(name="psum_s", bufs=4, space="PSUM"))

        # --- constants ---
        ones_kk = persist.tile([128, 128], b16, tag="ones_kk")
        nc.vector.memset(ones_kk, 1.0)
        bdiag = persist.tile([128, 128], b16, tag="bdiag")
        nc.sync.dma_start(out=bdiag, in_=bdiag_e[:, :])
        rew = persist.tile([MC, J], f32, tag="rew")
        nc.sync.dma_start(out=rew, in_=rew_e[:, :])
        term = persist.tile([MC, J], f32, tag="term")
        nc.sync.dma_start(out=term, in_=term_e[:, :])
        pol_cols = persist.tile([128, NG], f32, tag="pol_cols")
        ent_cols = persist.tile([128, NG], f32, tag="ent_cols")

        ptiles = {}
        for nk, _ in _NETS:
            for l in range(NLAYERS):
                pt = persist.tile([128, 12], f32, tag=f"p_{nk}_{l}")
                nc.sync.dma_start(out=pt, in_=p_e[(nk, l)][:, :])
                pt2 = persist.tile([128, 12], f32, name=f"p2_{nk}_{l}",
                                   tag=f"p2_{nk}_{l}")
                nc.scalar.copy(pt2, pt)
                ptiles[(nk, l)] = pt2

        def load_weights(nk):
            """DMA one net's weights into SBUF chunk tiles."""
            ws = {}
            for l in range(NLAYERS):
                ksz = KSZ_L0 if l == 0 else KSZ_H
                kofs = 0
                for ki, kn in enumerate(ksz):
                    wt = wpool.tile([kn, HID], b16, tag=f"w_l{l}_k{ki}")
                    nc.sync.dma_start(out=wt, in_=w_e[(nk, l)][kofs:kofs + kn, :])
                    ws[(l, ki)] = wt
                    kofs += kn
            dout = dict(_NETS)[nk]
            kofs = 0
            for ki, kn in enumerate(KSZ_F):
                wt = wpool.tile([kn, dout], b16, tag=f"wf_k{ki}")
                nc.sync.dma_start(out=wt, in_=wf_e[nk][kofs:kofs + kn, :])
                ws[("f", ki)] = wt
                kofs += kn
            return ws

        def stream_x0(g):
            gofs = g * GW
            xin = []
            for ki, kn in enumerate(KSZ_L0):
                xt = x0pool.tile([kn, GW], b16, name=f"x0_{ki}_{g % 2}",
                                 tag=f"x0_{ki}_{g % 2}")
                nc.sync.dma_start(out=xt, in_=x0_e[ki * 128:ki * 128 + kn, gofs:gofs + GW])
                xin.append(xt)
            return xin

        def emit_layer(nk, ws, l, g, cur):
            s = g % 2   # stream id for tag separation
            ksz = KSZ_L0 if l == 0 else KSZ_H
            pt = ptiles[(nk, l)]
            ct = [ctp.tile([CSZ[c], GW], b16, name=f"ct{c}_{s}", tag=f"ct{c}_{s}")
                  for c in range(4)]
            lnv = mid.tile([128, GW], b16, name=f"lnv_{s}", tag=f"lnv_{s}")
            sqs = {}
            for t in range(NT):
                tsl = slice(t * TW, (t + 1) * TW)
                for c in range(4):
                    csz = CSZ[c]
                    pc = psum_c.tile([csz, TW], f32, name="pc", tag="pc")
                    for ki, kn in enumerate(ksz):
                        nc.tensor.matmul(
                            pc, lhsT=ws[(l, ki)][:, COFS[c]:COFS[c] + csz],
                            rhs=cur[ki][0:kn, tsl],
                            start=(ki == 0), stop=(ki == len(ksz) - 1))
                    # PSUM -> SBUF cast with centered-bias add; split ACT/DVE
                    dst = ct[c][:, tsl]
                    if c < 2:
                        nc.scalar.activation(dst, pc, AF.Identity,
                                             bias=pt[0:csz, 3 * c:3 * c + 1],
                                             scale=1.0)
                    else:
                        nc.vector.tensor_scalar(dst, pc,
                                                pt[0:csz, 3 * c:3 * c + 1],
                                                None, OP.add)
                    # square on gpsimd, feeding the variance stats matmul
                    sqt = small.tile([csz, TW], b16, name=f"sq_{s}",
                                     tag=f"sq_{s}")
                    nc.gpsimd.tensor_mul(sqt, dst, dst)
                    sqs[(t, c)] = sqt
                ps = psum_s.tile([128, TW], f32, name="ps", tag="ps")
                for c in range(4):
                    nc.tensor.matmul(ps, lhsT=ones_kk[0:CSZ[c], :],
                                     rhs=sqs[(t, c)],
                                     start=(c == 0), stop=(c == 3))
                nc.scalar.activation(lnv[:, tsl], ps, AF.Ln,
                                     scale=1.0 / (HID * LN_EPS), bias=1.0)
            # rg = (var/eps + 1)^-0.5 = rstd * sqrt(eps); the eps^-0.5 and g
            # factors are folded into the v-scale param column
            rg = mid.tile([128, GW], b16, name=f"rg_{s}", tag=f"rg_{s}")
            nc.scalar.activation(rg, lnv, AF.Exp, scale=-0.5)
            nxt = []
            for c in range(4):
                csz = CSZ[c]
                uu = mid.tile([csz, GW], b16, name=f"uu_{s}", tag=f"uu_{s}")
                nc.vector.tensor_mul(uu, ct[c], rg[0:csz, :])
                vv = mid.tile([csz, GW], b16, name=f"vv_{s}", tag=f"vv_{s}")
                nc.vector.tensor_scalar(vv, uu, pt[0:csz, 3 * c + 1:3 * c + 2],
                                        pt[0:csz, 3 * c + 2:3 * c + 3],
                                        OP.mult, OP.add)
                ee = mid.tile([csz, GW], b16, name=f"ee_{s}", tag=f"lnv_{s}")
                nc.scalar.activation(ee, vv, AF.Exp)
                # elu(v) = max(v, min(e^v, 1) - 1)  (exp-first, overflow-safe)
                tt = mid.tile([csz, GW], b16, name=f"tt_{s}", tag=f"uu_{s}")
                nc.vector.tensor_scalar(tt, ee, 1.0, 1.0, OP.min, OP.subtract)
                xc = xpool.tile([csz + (1 if c == 3 else 0), GW], b16,
                                name=f"xn{c}_{s}", tag=f"xn{c}_{s}")
                if c == 3 and l == NLAYERS - 1:
                    # row 16 doubles as the ones row feeding the head's
                    # bias matmul chunk; partition-16 writes are illegal
                    # so fill the whole tile first.
                    nc.gpsimd.memset(xc, 1.0)
                nc.vector.tensor_max(xc[0:csz, :], vv, tt)
                nxt.append(xc)
            return nxt

        def value_head(nk, ws, g, x4):
            for t in range(NT):
                tsl = slice(t * TW, (t + 1) * TW)
                pv = psum_c.tile([1, TW], f32, tag="pc")
                for ki, kn in enumerate(KSZ_F):
                    nc.tensor.matmul(pv, lhsT=ws[("f", ki)], rhs=x4[ki][0:kn, tsl],
                                     start=(ki == 0), stop=(ki == 3))
                vtmp = small.tile([1, TW], f32, tag="vtmp")
                nc.scalar.copy(vtmp, pv)
                nc.sync.dma_start(out=vd[nk][g * NT + t:g * NT + t + 1, :], in_=vtmp)

        # ---- critic + target nets (two token groups interleaved so PE
        # works on one group's matmuls during the other's elementwise tail) ----
        for nk in ("c", "t"):
            ws = load_weights(nk)
            for g0 in (0, 2):
                curs = {g: stream_x0(g) for g in (g0, g0 + 1)}
                for l in range(NLAYERS):
                    for g in (g0, g0 + 1):
                        curs[g] = emit_layer(nk, ws, l, g, curs[g])
                for g in (g0, g0 + 1):
                    value_head(nk, ws, g, curs[g])

        # ---- GAE / reality weight / value loss ----
        import concourse.bass as bass_mod
        v_sb = persist.tile([MC, J], f32, tag="v_sb")
        vt_sb = persist.tile([MC, J], f32, tag="vt_sb")
        for nk, dst in (("c", v_sb), ("t", vt_sb)):
            src = bass_mod.AP(tensor=vd[nk][:, :].tensor, offset=0,
                              ap=[[J, MC], [1, J]])
            nc.sync.dma_start(out=dst, in_=src)

        gae = persist
        q = gae.tile([MC, H], f32, tag="q")
        nc.vector.tensor_scalar(q, term[:, 0:H], -GAMMA, GAMMA, OP.mult, OP.add)
        s1 = gae.tile([MC, H], f32, tag="s1")
        nc.vector.tensor_mul(s1, q, vt_sb[:, 0:H])
        s2 = gae.tile([MC, H], f32, tag="s2")
        nc.vector.tensor_sub(s2, rew[:, 0:H], vt_sb[:, 1:J])
        adv = gae.tile([MC, H], f32, tag="adv")
        nc.vector.tensor_add(adv, s1, s2)
        cl = gae.tile([MC, H], f32, tag="cl")
        nc.vector.tensor_scalar(cl, q, LAMBDA, None, OP.mult)
        agae = gae.tile([MC, H], f32, tag="agae")
        nc.vector.tensor_tensor_scan(agae, cl, adv, 0.0, OP.mult, OP.add)
        vtar = gae.tile([MC, H], f32, tag="vtar")
        nc.vector.tensor_add(vtar, agae, vt_sb[:, 1:J])
        delta = gae.tile([MC, H], f32, tag="delta")
        nc.vector.tensor_sub(delta, vtar, v_sb[:, 1:J])

        lg = gae.tile([MC, H], f32, tag="lg")
        nc.scalar.activation(lg, term[:, 1:J], AF.Ln, scale=-1.0, bias=1.0)
        S = gae.tile([MC, 1], f32, tag="S")
        nc.vector.tensor_reduce(S, lg, axis=mybir.AxisListType.X, op=OP.add)
        ones_mc = gae.tile([MC, H], f32, tag="ones_mc")
        nc.vector.memset(ones_mc, 1.0)
        cinc = gae.tile([MC, H], f32, tag="cinc")
        nc.vector.tensor_tensor_scan(cinc, ones_mc, lg, 0.0, OP.mult, OP.add)
        cx = gae.tile([MC, H], f32, tag="cx")
        nc.vector.tensor_sub(cx, cinc, lg)
        rwf = gae.tile([MC, J], f32, tag="rwf")
        nc.vector.memset(rwf[:, 0:1], 0.0)
        nc.scalar.activation(rwf[:, 1:J], cx, AF.Exp, scale=-1.0, bias=S)
        gfull = gae.tile([MC, J], f32, tag="gfull")
        nc.vector.memset(gfull[:, 0:1], 0.0)
        nc.vector.tensor_mul(gfull[:, 1:J], agae, rwf[:, 1:J])

        d2 = gae.tile([MC, H], f32, tag="d2")
        nc.vector.tensor_mul(d2, delta, delta)
        val_acc = gae.tile([MC, 1], f32, tag="val_acc")
        scrap64 = gae.tile([MC, H], f32, tag="scrap64")
        nc.vector.scalar_tensor_tensor(scrap64, d2, 1.0, rwf[:, 1:J],
                                       OP.mult, OP.mult, accum_out=val_acc)
        nc.sync.dma_start(out=out_e[0:MC, 0:1], in_=val_acc)

        gdst = bass_mod.AP(tensor=gbuf[:].tensor, offset=0, ap=[[J, MC], [1, J]])
        rwdst = bass_mod.AP(tensor=rwbuf[:].tensor, offset=0, ap=[[J, MC], [1, J]])
        nc.sync.dma_start(out=gdst, in_=gfull)
        nc.sync.dma_start(out=rwdst, in_=rwf)

        # ---- actor net + policy head ----
        ws = load_weights("a")
        x4s = {}
        for g0 in (0, 2):
            curs = {g: stream_x0(g) for g in (g0, g0 + 1)}
            for l in range(NLAYERS):
                for g in (g0, g0 + 1):
                    curs[g] = emit_layer("a", ws, l, g, curs[g])
            x4s.update(curs)
        for g in range(NG):
            x4 = x4s[g]
            pz = psum_s.tile([128, TW], f32, tag="ps")
            for b in range(4):
                bsl = slice(b * TW, (b + 1) * TW)
                for ki, kn in enumerate(KSZ_F):
                    nc.tensor.matmul(pz[32 * b:32 * b + 32, :], lhsT=ws[("f", ki)],
                                     rhs=x4[ki][0:kn, bsl],
                                     start=(ki == 0), stop=(ki == 3),
                                     tile_position=(0, 32 * b))
            a_sb = headp.tile([128, TW], b16, tag="a_sb")
            nc.sync.dma_start(out=a_sb, in_=act_e[g])
            E = headp.tile([128, TW], b16, tag="E")
            nc.scalar.activation(E, pz, AF.Exp)
            Ez = headp.tile([128, TW], b16, tag="Ez")
            nc.vector.tensor_mul(Ez, pz, E)
            az = headp.tile([128, TW], b16, tag="az")
            nc.vector.tensor_mul(az, pz, a_sb)
            sums = {}
            for qn, src in (("S", E), ("W", Ez), ("U", az), ("B", a_sb)):
                p = psum_s.tile([128, TW], f32, tag="ps")
                nc.tensor.matmul(p, lhsT=bdiag, rhs=src, start=True, stop=True)
                sums[qn] = p
            lnS = headp.tile([128, TW], f32, tag="lnS")
            nc.scalar.activation(lnS, sums["S"], AF.Ln)
            Sinv = headp.tile([128, TW], b16, tag="Sinv")
            nc.scalar.activation(Sinv, lnS, AF.Exp, scale=-1.0)
            t1 = headp.tile([128, TW], b16, tag="t1")
            nc.vector.tensor_mul(t1, sums["B"], lnS)
            alp = headp.tile([128, TW], b16, tag="alp")
            nc.vector.tensor_sub(alp, sums["U"], t1)
            t2 = headp.tile([128, TW], b16, tag="t2")
            nc.vector.tensor_mul(t2, sums["W"], Sinv)
            ent = headp.tile([128, TW], b16, tag="ent")
            nc.vector.tensor_sub(ent, lnS, t2)

            gstk = headp.tile([128, TW], f32, tag="gstk")
            gsrc = bass_mod.AP(tensor=gbuf[:].tensor, offset=g * GW,
                               ap=[[TW, 4], [0, 32], [1, TW]])
            nc.gpsimd.dma_start(out=gstk, in_=gsrc)
            rstk = headp.tile([128, TW], f32, tag="rstk")
            rsrc = bass_mod.AP(tensor=rwbuf[:].tensor, offset=g * GW,
                               ap=[[TW, 4], [0, 32], [1, TW]])
            nc.gpsimd.dma_start(out=rstk, in_=rsrc)
            scrap = headp.tile([128, TW], b16, tag="scrap")
            nc.vector.scalar_tensor_tensor(scrap, alp, 1.0, gstk, OP.mult, OP.mult,
                                           accum_out=pol_cols[:, g:g + 1])
            scrap2 = headp.tile([128, TW], b16, tag="scrap2")
            nc.vector.scalar_tensor_tensor(scrap2, ent, 1.0, rstk, OP.mult, OP.mult,
                                           accum_out=ent_cols[:, g:g + 1])

        pol_sum = persist.tile([128, 1], f32, tag="pol_sum")
        nc.vector.tensor_reduce(pol_sum, pol_cols, axis=mybir.AxisListType.X, op=OP.add)
        ent_sum = persist.tile([128, 1], f32, tag="ent_sum")
        nc.vector.tensor_reduce(ent_sum, ent_cols, axis=mybir.AxisListType.X, op=OP.add)
        nc.sync.dma_start(out=out_e[:, 1:2], in_=pol_sum)
        nc.sync.dma_start(out=out_e[:, 2:3], in_=ent_sum)

    import concourse.bacc as bacc_mod
    import concourse.hw_specs as hw_specs
    orig_tables = hw_specs.get_activation_tables
    keep = "natural_log_exp_and_others"
    mine = {AF.Identity, AF.Copy, AF.Exp, AF.Ln, AF.Square}

    def patched_tables(arch):
        out = {}
        for name, s in orig_tables(arch).items():
            out[name] = s if name == keep else (s - mine)
        return out

    bacc_mod.get_activation_tables = patched_tables
    try:
        nc.compile()
    finally:
        bacc_mod.get_activation_tables = orig_tables
    return nc


_NC_CACHE = {}


def kernel(features, rewards_mean, terminals_mean, actions,
           actor_params, critic_params, target_params, _want_timing=False):
    from concourse.bass_utils import run_bass_kernel_spmd

    in_maps = prep_all(features, rewards_mean, terminals_mean, actions,
                       actor_params, critic_params, target_params)
    if "nc" not in _NC_CACHE:
        _NC_CACHE["nc"] = build_graph()
    nc = _NC_CACHE["nc"]
    res = run_bass_kernel_spmd(nc, in_maps, core_ids=list(range(NCORES)),
                               trace=_want_timing)
    outs = [np.asarray(r["out"], np.float64) for r in res.results]
    val = sum(o[0:MC, 0].sum() for o in outs)
    pol = sum(o[:, 1].sum() for o in outs) / 32.0
    ent = sum(o[:, 2].sum() for o in outs) / 32.0
    denom = float(H * M)
    loss_value = 0.5 * val / denom
    loss_policy = -pol / denom
    policy_entropy = ent / denom
    loss_actor = loss_policy - TEMPERATURE * policy_entropy
    out = np.array([loss_actor, loss_value], np.float32)
    if _want_timing:
        return out, res
    return out


# revision 24
# speedup vs baseline: 1.2970x; 1.0145x over previous
"""Trainium2 Bass kernel for nn_ActorCritic loss (8-core SPMD, data-parallel over batch).

Strategy:
- Shard batch M=512 over 8 cores (64 each). MLP params replicated.
- Host prep: features pre-transposed to feature-major [F, T] per core with token
  column t = m*J + u, u = J-1-j (time-reversed so the GAE recursion becomes a
  forward hardware scan). LayerNorm mean-subtraction is folded into the weights
  (W - rowmean, b - mean); LN gain g is folded into the rstd computation via
  exp(-0.5*ln(var+eps) + ln g).
- On chip per core: 3 MLPs (critic/target/actor) in feature-major layout, bf16
  matmuls with fp32 PSUM. Per-token variance via ones-matrix matmul broadcast.
  ELU via exp/min/max with scalar_tensor_tensor fusion. GAE via
  tensor_tensor_scan. Reality weight via suffix-cumsum trick (no reversal).
  Actor head: stacked [4x32, 512] logits tiles, block-diagonal ones matmul for
  per-token reductions over A.
- Each core emits partial loss sums; host combines to the final [2] output.
"""
import os
import sys
import numpy as np

for _p in ("/opt/trn_rl_repo",):
    if _p not in sys.path and os.path.isdir(_p):
        sys.path.insert(0, _p)

import ml_dtypes  # noqa: E402

bf16 = ml_dtypes.bfloat16

GAMMA = 0.999
LAMBDA = 0.95
TEMPERATURE = 0.001
J, M, F, A = 128, 512, 256, 32
H = J - 1
HID, NLAYERS = 400, 4
LN_EPS = 1e-3
NCORES = 8
MC = M // NCORES          # 64 batch elements per core
T = MC * J                # 8192 tokens per core
NG = 4                    # token groups
GW = T // NG              # 2048 cols per group
NT = 4                    # 512-col tiles per group
TW = GW // NT             # 512

CSZ = [128, 128, 128, 16]         # dout chunks of HID=400
COFS = [0, 128, 256, 384]
KSZ_L0 = [128, 128]               # din chunks of F=256
KSZ_H = [128, 128, 128, 16]       # din chunks of HID=400
KSZ_F = [128, 128, 128, 17]       # din chunks of HID+1 (bias row)

_NETS = [("c", 1), ("t", 1), ("a", 32)]


# ----------------------------------------------------------------------------
# host-side prep
# ----------------------------------------------------------------------------

def _prep_net(params):
    hidden, (Wf, bf_) = params
    pre = []
    for (W, b, g, beta) in hidden:
        W = np.asarray(W, np.float32)
        b = np.asarray(b, np.float32)
        g = np.asarray(g, np.float32)
        beta = np.asarray(beta, np.float32)
        Wc = W - W.mean(axis=1, keepdims=True)
        bc = b - b.mean()
        if np.any(g <= 0):
            raise ValueError("LN gain fold requires g > 0")
        pre.append((Wc.astype(bf16), bc, np.log(g), beta))
    Wf = np.asarray(Wf, np.float32)
    bf_ = np.asarray(bf_, np.float32)
    Wf_aug = np.concatenate([Wf, bf_[None, :]], axis=0).astype(bf16)
    return pre, Wf_aug


def _param_tile(pre_l):
    """[128, 12] fp32: col 3c+0 = centered bias, 3c+1 = ln g, 3c+2 = beta for chunk c."""
    _, bc, lng, beta = pre_l
    out = np.zeros((128, 12), np.float32)
    for c in range(4):
        n = CSZ[c]
        sl = slice(COFS[c], COFS[c] + n)
        out[:n, 3 * c + 0] = bc[sl]
        out[:n, 3 * c + 1] = np.exp(lng[sl]) * LN_EPS ** -0.5
        out[:n, 3 * c + 2] = beta[sl]
    return out


def prep_all(features, rewards_mean, terminals_mean, actions,
             actor_params, critic_params, target_params):
    features = np.asarray(features, np.float32)
    rewards_mean = np.asarray(rewards_mean, np.float32)
    terminals_mean = np.asarray(terminals_mean, np.float32)
    actions = np.asarray(actions, np.float32)

    nets = {}
    for key, params in (("c", critic_params), ("t", target_params), ("a", actor_params)):
        nets[key] = _prep_net(params)

    shared = {"bdiag": np.kron(np.eye(4, dtype=np.float32),
                               np.ones((32, 32), np.float32)).astype(bf16)}
    for nk, (pre, Wf_aug) in nets.items():
        shared[f"wf_{nk}"] = np.ascontiguousarray(Wf_aug)
        for l, p in enumerate(pre):
            shared[f"w_{nk}_{l}"] = np.ascontiguousarray(p[0])
            shared[f"p_{nk}_{l}"] = _param_tile(p)

    in_maps = []
    for core in range(NCORES):
        msl = slice(core * MC, (core + 1) * MC)
        d = dict(shared)
        x0 = features[::-1, msl, :].transpose(2, 1, 0).reshape(F, T)
        d["x0"] = np.ascontiguousarray(x0.astype(bf16))
        d["rew"] = np.ascontiguousarray(rewards_mean[::-1, msl].T)
        d["term"] = np.ascontiguousarray(terminals_mean[::-1, msl].T)
        act_u = np.zeros((J, MC, A), np.float32)
        act_u[1:] = actions[::-1, msl, :]
        act_T = act_u.transpose(2, 1, 0).reshape(A, T)          # [32, 8192]
        act_stk = act_T.reshape(A, NG, NT, TW).transpose(1, 2, 0, 3).reshape(NG, 128, TW)
        d["act_stk"] = np.ascontiguousarray(act_stk.astype(bf16))
        in_maps.append(d)
    return in_maps


# ----------------------------------------------------------------------------
# bass graph
# ----------------------------------------------------------------------------

def build_graph():
    import contextlib
    import concourse.bass as bass
    import concourse.tile as tile
    from concourse import bacc, mybir

    f32 = mybir.dt.float32
    b16 = mybir.dt.bfloat16
    AF = mybir.ActivationFunctionType
    OP = mybir.AluOpType

    nc = bacc.Bacc()

    x0_e = nc.declare_dram_parameter("x0", [F, T], b16, isOutput=False)
    rew_e = nc.declare_dram_parameter("rew", [MC, J], f32, isOutput=False)
    term_e = nc.declare_dram_parameter("term", [MC, J], f32, isOutput=False)
    act_e = nc.declare_dram_parameter("act_stk", [NG, 128, TW], b16, isOutput=False)
    bdiag_e = nc.declare_dram_parameter("bdiag", [128, 128], b16, isOutput=False)
    w_e, p_e, wf_e = {}, {}, {}
    for nk, dout in _NETS:
        wf_e[nk] = nc.declare_dram_parameter(f"wf_{nk}", [HID + 1, dout], b16, isOutput=False)
        for l in range(NLAYERS):
            din = F if l == 0 else HID
            w_e[(nk, l)] = nc.declare_dram_parameter(f"w_{nk}_{l}", [din, HID], b16, isOutput=False)
            p_e[(nk, l)] = nc.declare_dram_parameter(f"p_{nk}_{l}", [128, 12], f32, isOutput=False)
    out_e = nc.declare_dram_parameter("out", [128, 8], f32, isOutput=True)

    vd = {nk: nc.dram_tensor(f"vd_{nk}", [NG * NT, TW], f32) for nk in ("c", "t")}
    gbuf = nc.dram_tensor("gbuf", [T], f32)
    rwbuf = nc.dram_tensor("rwbuf", [T], f32)

    with tile.TileContext(nc) as tc, contextlib.ExitStack() as ctx:
        persist = ctx.enter_context(tc.tile_pool(name="persist", bufs=1))
        wpool = ctx.enter_context(tc.tile_pool(name="wpool", bufs=1))
        xpool = ctx.enter_context(tc.tile_pool(name="xpool", bufs=2))
        x0pool = ctx.enter_context(tc.tile_pool(name="x0pool", bufs=1))
        mid = ctx.enter_context(tc.tile_pool(name="mid", bufs=1))
        ctp = ctx.enter_context(tc.tile_pool(name="ctp", bufs=1))
        small = ctx.enter_context(tc.tile_pool(name="small", bufs=2))
        headp = ctx.enter_context(tc.tile_pool(name="headp", bufs=1))
        psum_c = ctx.enter_context(tc.tile_pool(name="psum_c", bufs=4, space="PSUM"))
        psum_s = ctx.enter_context(tc.tile_pool(name="psum_s", bufs=4, space="PSUM"))

        # --- constants ---
        ones_kk = persist.tile([128, 128], b16, tag="ones_kk")
        nc.vector.memset(ones_kk, 1.0)
        bdiag = persist.tile([128, 128], b16, tag="bdiag")
        nc.sync.dma_start(out=bdiag, in_=bdiag_e[:, :])
        rew = persist.tile([MC, J], f32, tag="rew")
        nc.sync.dma_start(out=rew, in_=rew_e[:, :])
        term = persist.tile([MC, J], f32, tag="term")
        nc.sync.dma_start(out=term, in_=term_e[:, :])
        pol_cols = persist.tile([128, NG], f32, tag="pol_cols")
        ent_cols = persist.tile([128, NG], f32, tag="ent_cols")

        ptiles = {}
        for nk, _ in _NETS:
            for l in range(NLAYERS):
                pt = persist.tile([128, 12], f32, tag=f"p_{nk}_{l}")
                nc.sync.dma_start(out=pt, in_=p_e[(nk, l)][:, :])
                pt2 = persist.tile([128, 12], f32, name=f"p2_{nk}_{l}",
                                   tag=f"p2_{nk}_{l}")
                nc.scalar.copy(pt2, pt)
                ptiles[(nk, l)] = pt2

        def load_weights(nk):
            """DMA one net's weights into SBUF chunk tiles."""
            ws = {}
            for l in range(NLAYERS):
                ksz = KSZ_L0 if l == 0 else KSZ_H
                kofs = 0
                for ki, kn in enumerate(ksz):
                    wt = wpool.tile([kn, HID], b16, tag=f"w_l{l}_k{ki}")
                    nc.sync.dma_start(out=wt, in_=w_e[(nk, l)][kofs:kofs + kn, :])
                    ws[(l, ki)] = wt
                    kofs += kn
            dout = dict(_NETS)[nk]
            kofs = 0
            for ki, kn in enumerate(KSZ_F):
                wt = wpool.tile([kn, dout], b16, tag=f"wf_k{ki}")
                nc.sync.dma_start(out=wt, in_=wf_e[nk][kofs:kofs + kn, :])
                ws[("f", ki)] = wt
                kofs += kn
            return ws

        def stream_x0(g):
            gofs = g * GW
            xin = []
            for ki, kn in enumerate(KSZ_L0):
                xt = x0pool.tile([kn, GW], b16, name=f"x0_{ki}_{g % 2}",
                                 tag=f"x0_{ki}_{g % 2}")
                nc.sync.dma_start(out=xt, in_=x0_e[ki * 128:ki * 128 + kn, gofs:gofs + GW])
                xin.append(xt)
            return xin

        def emit_layer_pair(nk, ws, l, streams):
            """Emit one layer for two token groups, interleaved at tile
            granularity with 1-tile-delayed stats matmuls so PE never waits
            on the cast->square chain."""
            ksz = KSZ_L0 if l == 0 else KSZ_H
            pt = ptiles[(nk, l)]
            st = {}
            for g, cur in streams.items():
                s = g % 2
                st[g] = dict(
                    s=s, cur=cur,
                    ct=[ctp.tile([CSZ[c], GW], b16, name=f"ct{c}_{s}",
                                 tag=f"ct{c}_{s}") for c in range(4)],
                    lnv=mid.tile([128, GW], b16, name=f"lnv_{s}", tag=f"lnv_{s}"),
                    sqs={},
                )

            def mm_tile(g, t):
                d = st[g]
                tsl = slice(t * TW, (t + 1) * TW)
                for c in range(4):
                    csz = CSZ[c]
                    pc = psum_c.tile([csz, TW], f32, name="pc", tag="pc")
                    for ki, kn in enumerate(ksz):
                        nc.tensor.matmul(
                            pc, lhsT=ws[(l, ki)][:, COFS[c]:COFS[c] + csz],
                            rhs=d["cur"][ki][0:kn, tsl],
                            start=(ki == 0), stop=(ki == len(ksz) - 1))
                    dst = d["ct"][c][:, tsl]
                    if c < 3:
                        nc.scalar.activation(dst, pc, AF.Identity,
                                             bias=pt[0:csz, 3 * c:3 * c + 1],
                                             scale=1.0)
                    else:
                        nc.vector.tensor_scalar(dst, pc,
                                                pt[0:csz, 3 * c:3 * c + 1],
                                                None, OP.add)
                    sqt = small.tile([csz, TW], b16, name=f"sq{c}_{d['s']}",
                                     tag=f"sq{c}_{d['s']}")
                    nc.gpsimd.tensor_mul(sqt, dst, dst)
                    d["sqs"][(t, c)] = sqt

            def stats_tile(g, t):
                d = st[g]
                tsl = slice(t * TW, (t + 1) * TW)
                ps = psum_s.tile([128, TW], f32, name="ps", tag="ps")
                for c in range(4):
                    nc.tensor.matmul(ps, lhsT=ones_kk[0:CSZ[c], :],
                                     rhs=d["sqs"].pop((t, c)),
                                     start=(c == 0), stop=(c == 3))
                nc.scalar.activation(d["lnv"][:, tsl], ps, AF.Ln,
                                     scale=1.0 / (HID * LN_EPS), bias=1.0)

            gs = list(streams)
            pending = []
            for t in range(NT):
                for g in gs:
                    mm_tile(g, t)
                    pending.append((g, t))
                    if len(pending) > 2:
                        stats_tile(*pending.pop(0))
            for item in pending:
                stats_tile(*item)

            out = {}
            for g in gs:
                d = st[g]
                s = d["s"]
                # rg = (var/eps + 1)^-0.5 = rstd*sqrt(eps); eps^-0.5 and g
                # factors are folded into the v-scale param column
                rg = mid.tile([128, GW], b16, name=f"rg_{s}", tag=f"rg_{s}")
                nc.scalar.activation(rg, d["lnv"], AF.Exp, scale=-0.5)
                nxt = []
                for c in range(4):
                    csz = CSZ[c]
                    uu = mid.tile([csz, GW], b16, name=f"uu_{s}", tag=f"uu_{s}")
                    nc.vector.tensor_mul(uu, d["ct"][c], rg[0:csz, :])
                    vv = mid.tile([csz, GW], b16, name=f"vv_{s}", tag=f"vv_{s}")
                    nc.vector.tensor_scalar(vv, uu, pt[0:csz, 3 * c + 1:3 * c + 2],
                                            pt[0:csz, 3 * c + 2:3 * c + 3],
                                            OP.mult, OP.add)
                    ee = mid.tile([csz, GW], b16, name=f"ee_{s}", tag=f"lnv_{s}")
                    nc.scalar.activation(ee, vv, AF.Exp)
                    # elu(v) = max(v, min(e^v, 1) - 1)  (exp-first, overflow-safe)
                    tt = mid.tile([csz, GW], b16, name=f"tt_{s}", tag=f"uu_{s}")
                    nc.vector.tensor_scalar(tt, ee, 1.0, 1.0, OP.min, OP.subtract)
                    xc = xpool.tile([csz + (1 if c == 3 else 0), GW], b16,
                                    name=f"xn{c}_{s}", tag=f"xn{c}_{s}")
                    if c == 3 and l == NLAYERS - 1:
                        # row 16 doubles as the ones row feeding the head's
                        # bias matmul chunk; partition-16 writes are illegal
                        # so fill the whole tile first.
                        nc.gpsimd.memset(xc, 1.0)
                    nc.vector.tensor_max(xc[0:csz, :], vv, tt)
                    nxt.append(xc)
                out[g] = nxt
            return out

        def value_head(nk, ws, g, x4):
            for t in range(NT):
                tsl = slice(t * TW, (t + 1) * TW)
                pv = psum_c.tile([1, TW], f32, tag="pc")
                for ki, kn in enumerate(KSZ_F):
                    nc.tensor.matmul(pv, lhsT=ws[("f", ki)], rhs=x4[ki][0:kn, tsl],
                                     start=(ki == 0), stop=(ki == 3))
                vtmp = small.tile([1, TW], f32, tag="vtmp")
                nc.scalar.copy(vtmp, pv)
                nc.sync.dma_start(out=vd[nk][g * NT + t:g * NT + t + 1, :], in_=vtmp)

        # ---- critic + target nets (two token groups interleaved so PE
        # works on one group's matmuls during the other's elementwise tail) ----
        for nk in ("c", "t"):
            ws = load_weights(nk)
            for g0 in (0, 2):
                curs = {g: stream_x0(g) for g in (g0, g0 + 1)}
                for l in range(NLAYERS):
                    curs = emit_layer_pair(nk, ws, l, curs)
                for g in (g0, g0 + 1):
                    value_head(nk, ws, g, curs[g])

        # ---- GAE / reality weight / value loss ----
        import concourse.bass as bass_mod
        v_sb = persist.tile([MC, J], f32, tag="v_sb")
        vt_sb = persist.tile([MC, J], f32, tag="vt_sb")
        for nk, dst in (("c", v_sb), ("t", vt_sb)):
            src = bass_mod.AP(tensor=vd[nk][:, :].tensor, offset=0,
                              ap=[[J, MC], [1, J]])
            nc.sync.dma_start(out=dst, in_=src)

        gae = persist
        q = gae.tile([MC, H], f32, tag="q")
        nc.vector.tensor_scalar(q, term[:, 0:H], -GAMMA, GAMMA, OP.mult, OP.add)
        s1 = gae.tile([MC, H], f32, tag="s1")
        nc.vector.tensor_mul(s1, q, vt_sb[:, 0:H])
        s2 = gae.tile([MC, H], f32, tag="s2")
        nc.vector.tensor_sub(s2, rew[:, 0:H], vt_sb[:, 1:J])
        adv = gae.tile([MC, H], f32, tag="adv")
        nc.vector.tensor_add(adv, s1, s2)
        cl = gae.tile([MC, H], f32, tag="cl")
        nc.vector.tensor_scalar(cl, q, LAMBDA, None, OP.mult)
        agae = gae.tile([MC, H], f32, tag="agae")
        nc.vector.tensor_tensor_scan(agae, cl, adv, 0.0, OP.mult, OP.add)
        vtar = gae.tile([MC, H], f32, tag="vtar")
        nc.vector.tensor_add(vtar, agae, vt_sb[:, 1:J])
        delta = gae.tile([MC, H], f32, tag="delta")
        nc.vector.tensor_sub(delta, vtar, v_sb[:, 1:J])

        lg = gae.tile([MC, H], f32, tag="lg")
        nc.scalar.activation(lg, term[:, 1:J], AF.Ln, scale=-1.0, bias=1.0)
        S = gae.tile([MC, 1], f32, tag="S")
        nc.vector.tensor_reduce(S, lg, axis=mybir.AxisListType.X, op=OP.add)
        ones_mc = gae.tile([MC, H], f32, tag="ones_mc")
        nc.vector.memset(ones_mc, 1.0)
        cinc = gae.tile([MC, H], f32, tag="cinc")
        nc.vector.tensor_tensor_scan(cinc, ones_mc, lg, 0.0, OP.mult, OP.add)
        cx = gae.tile([MC, H], f32, tag="cx")
        nc.vector.tensor_sub(cx, cinc, lg)
        rwf = gae.tile([MC, J], f32, tag="rwf")
        nc.vector.memset(rwf[:, 0:1], 0.0)
        nc.scalar.activation(rwf[:, 1:J], cx, AF.Exp, scale=-1.0, bias=S)
        gfull = gae.tile([MC, J], f32, tag="gfull")
        nc.vector.memset(gfull[:, 0:1], 0.0)
        nc.vector.tensor_mul(gfull[:, 1:J], agae, rwf[:, 1:J])

        d2 = gae.tile([MC, H], f32, tag="d2")
        nc.vector.tensor_mul(d2, delta, delta)
        val_acc = gae.tile([MC, 1], f32, tag="val_acc")
        scrap64 = gae.tile([MC, H], f32, tag="scrap64")
        nc.vector.scalar_tensor_tensor(scrap64, d2, 1.0, rwf[:, 1:J],
                                       OP.mult, OP.mult, accum_out=val_acc)
        nc.sync.dma_start(out=out_e[0:MC, 0:1], in_=val_acc)

        gdst = bass_mod.AP(tensor=gbuf[:].tensor, offset=0, ap=[[J, MC], [1, J]])
        rwdst = bass_mod.AP(tensor=rwbuf[:].tensor, offset=0, ap=[[J, MC], [1, J]])
        nc.sync.dma_start(out=gdst, in_=gfull)
        nc.sync.dma_start(out=rwdst, in_=rwf)

        # ---- actor net + policy head ----
        ws = load_weights("a")
        x4s = {}
        for g0 in (0, 2):
            curs = {g: stream_x0(g) for g in (g0, g0 + 1)}
            for l in range(NLAYERS):
                curs = emit_layer_pair("a", ws, l, curs)
            x4s.update(curs)
        for g in range(NG):
            x4 = x4s[g]
            pz = psum_s.tile([128, TW], f32, tag="ps")
            for b in range(4):
                bsl = slice(b * TW, (b + 1) * TW)
                for ki, kn in enumerate(KSZ_F):
                    nc.tensor.matmul(pz[32 * b:32 * b + 32, :], lhsT=ws[("f", ki)],
                                     rhs=x4[ki][0:kn, bsl],
                                     start=(ki == 0), stop=(ki == 3),
                                     tile_position=(0, 32 * b))
            a_sb = headp.tile([128, TW], b16, tag="a_sb")
            nc.sync.dma_start(out=a_sb, in_=act_e[g])
            E = headp.tile([128, TW], b16, tag="E")
            nc.scalar.activation(E, pz, AF.Exp)
            Ez = headp.tile([128, TW], b16, tag="Ez")
            nc.vector.tensor_mul(Ez, pz, E)
            az = headp.tile([128, TW], b16, tag="az")
            nc.vector.tensor_mul(az, pz, a_sb)
            sums = {}
            for qn, src in (("S", E), ("W", Ez), ("U", az), ("B", a_sb)):
                p = psum_s.tile([128, TW], f32, tag="ps")
                nc.tensor.matmul(p, lhsT=bdiag, rhs=src, start=True, stop=True)
                sums[qn] = p
            lnS = headp.tile([128, TW], f32, tag="lnS")
            nc.scalar.activation(lnS, sums["S"], AF.Ln)
            Sinv = headp.tile([128, TW], b16, tag="Sinv")
            nc.scalar.activation(Sinv, lnS, AF.Exp, scale=-1.0)
            t1 = headp.tile([128, TW], b16, tag="t1")
            nc.vector.tensor_mul(t1, sums["B"], lnS)
            alp = headp.tile([128, TW], b16, tag="alp")
            nc.vector.tensor_sub(alp, sums["U"], t1)
            t2 = headp.tile([128, TW], b16, tag="t2")
            nc.vector.tensor_mul(t2, sums["W"], Sinv)
            ent = headp.tile([128, TW], b16, tag="ent")
            nc.vector.tensor_sub(ent, lnS, t2)

            gstk = headp.tile([128, TW], f32, tag="gstk")
            gsrc = bass_mod.AP(tensor=gbuf[:].tensor, offset=g * GW,
                               ap=[[TW, 4], [0, 32], [1, TW]])
            nc.gpsimd.dma_start(out=gstk, in_=gsrc)
            rstk = headp.tile([128, TW], f32, tag="rstk")
            rsrc = bass_mod.AP(tensor=rwbuf[:].tensor, offset=g * GW,
                               ap=[[TW, 4], [0, 32], [1, TW]])
            nc.gpsimd.dma_start(out=rstk, in_=rsrc)
            scrap = headp.tile([128, TW], b16, tag="scrap")
            nc.vector.scalar_tensor_tensor(scrap, alp, 1.0, gstk, OP.mult, OP.mult,
                                           accum_out=pol_cols[:, g:g + 1])
            scrap2 = headp.tile([128, TW], b16, tag="scrap2")
            nc.vector.scalar_tensor_tensor(scrap2, ent, 1.0, rstk, OP.mult, OP.mult,
                                           accum_out=ent_cols[:, g:g + 1])

        pol_sum = persist.tile([128, 1], f32, tag="pol_sum")
        nc.vector.tensor_reduce(pol_sum, pol_cols, axis=mybir.AxisListType.X, op=OP.add)
        ent_sum = persist.tile([128, 1], f32, tag="ent_sum")
        nc.vector.tensor_reduce(ent_sum, ent_cols, axis=mybir.AxisListType.X, op=OP.add)
        nc.sync.dma_start(out=out_e[:, 1:2], in_=pol_sum)
        nc.sync.dma_start(out=out_e[:, 2:3], in_=ent_sum)

    import concourse.bacc as bacc_mod
    import concourse.hw_specs as hw_specs
    orig_tables = hw_specs.get_activation_tables
    keep = "natural_log_exp_and_others"
    mine = {AF.Identity, AF.Copy, AF.Exp, AF.Ln, AF.Square}

    def patched_tables(arch):
        out = {}
        for name, s in orig_tables(arch).items():
            out[name] = s if name == keep else (s - mine)
        return out

    bacc_mod.get_activation_tables = patched_tables
    try:
        nc.compile()
    finally:
        bacc_mod.get_activation_tables = orig_tables
    return nc


_NC_CACHE = {}


def kernel(features, rewards_mean, terminals_mean, actions,
           actor_params, critic_params, target_params, _want_timing=False):
    from concourse.bass_utils import run_bass_kernel_spmd

    in_maps = prep_all(features, rewards_mean, terminals_mean, actions,
                       actor_params, critic_params, target_params)
    if "nc" not in _NC_CACHE:
        _NC_CACHE["nc"] = build_graph()
    nc = _NC_CACHE["nc"]
    res = run_bass_kernel_spmd(nc, in_maps, core_ids=list(range(NCORES)),
                               trace=_want_timing)
    outs = [np.asarray(r["out"], np.float64) for r in res.results]
    val = sum(o[0:MC, 0].sum() for o in outs)
    pol = sum(o[:, 1].sum() for o in outs) / 32.0
    ent = sum(o[:, 2].sum() for o in outs) / 32.0
    denom = float(H * M)
    loss_value = 0.5 * val / denom
    loss_policy = -pol / denom
    policy_entropy = ent / denom
    loss_actor = loss_policy - TEMPERATURE * policy_entropy
    out = np.array([loss_actor, loss_value], np.float32)
    if _want_timing:
        return out, res
    return out


# revision 26
# speedup vs baseline: 1.3768x; 1.0615x over previous
"""Trainium2 Bass kernel for nn_ActorCritic loss (8-core SPMD, data-parallel over batch).

Strategy:
- Shard batch M=512 over 8 cores (64 each). MLP params replicated.
- Host prep: features pre-transposed to feature-major [F, T] per core with token
  column t = m*J + u, u = J-1-j (time-reversed so the GAE recursion becomes a
  forward hardware scan). LayerNorm mean-subtraction is folded into the weights
  (W - rowmean, b - mean); LN gain g is folded into the rstd computation via
  exp(-0.5*ln(var+eps) + ln g).
- On chip per core: 3 MLPs (critic/target/actor) in feature-major layout, bf16
  matmuls with fp32 PSUM. Per-token variance via ones-matrix matmul broadcast.
  ELU via exp/min/max with scalar_tensor_tensor fusion. GAE via
  tensor_tensor_scan. Reality weight via suffix-cumsum trick (no reversal).
  Actor head: stacked [4x32, 512] logits tiles, block-diagonal ones matmul for
  per-token reductions over A.
- Each core emits partial loss sums; host combines to the final [2] output.
"""
import os
import sys
import numpy as np

for _p in ("/opt/trn_rl_repo",):
    if _p not in sys.path and os.path.isdir(_p):
        sys.path.insert(0, _p)

import ml_dtypes  # noqa: E402

bf16 = ml_dtypes.bfloat16

GAMMA = 0.999
LAMBDA = 0.95
TEMPERATURE = 0.001
J, M, F, A = 128, 512, 256, 32
H = J - 1
HID, NLAYERS = 400, 4
LN_EPS = 1e-3
NCORES = 8
MC = M // NCORES          # 64 batch elements per core
T = MC * J                # 8192 tokens per core
NG = 4                    # token groups
GW = T // NG              # 2048 cols per group
NT = 4                    # 512-col tiles per group
TW = GW // NT             # 512

CSZ = [128, 128, 128, 16]         # dout chunks of HID=400
COFS = [0, 128, 256, 384]
KSZ_L0 = [128, 128]               # din chunks of F=256
KSZ_H = [128, 128, 128, 16]       # din chunks of HID=400
KSZ_F = [128, 128, 128, 17]       # din chunks of HID+1 (bias row)

_NETS = [("c", 1), ("t", 1), ("a", 32)]


# ----------------------------------------------------------------------------
# host-side prep
# ----------------------------------------------------------------------------

def _prep_net(params):
    hidden, (Wf, bf_) = params
    pre = []
    for (W, b, g, beta) in hidden:
        W = np.asarray(W, np.float32)
        b = np.asarray(b, np.float32)
        g = np.asarray(g, np.float32)
        beta = np.asarray(beta, np.float32)
        Wc = W - W.mean(axis=1, keepdims=True)
        bc = b - b.mean()
        pre.append((Wc.astype(bf16), bc, g, beta))
    Wf = np.asarray(Wf, np.float32)
    bf_ = np.asarray(bf_, np.float32)
    Wf_aug = np.concatenate([Wf, bf_[None, :]], axis=0).astype(bf16)
    return pre, Wf_aug


def _param_tile(pre_l):
    """[128, 12] fp32: col 3c+0 = centered bias, 3c+1 = ln g, 3c+2 = beta for chunk c."""
    _, bc, g, beta = pre_l
    out = np.zeros((128, 12), np.float32)
    for c in range(4):
        n = CSZ[c]
        sl = slice(COFS[c], COFS[c] + n)
        out[:n, 3 * c + 0] = bc[sl]
        out[:n, 3 * c + 1] = g[sl] * LN_EPS ** -0.5
        out[:n, 3 * c + 2] = beta[sl]
    return out


def prep_all(features, rewards_mean, terminals_mean, actions,
             actor_params, critic_params, target_params):
    features = np.asarray(features, np.float32)
    rewards_mean = np.asarray(rewards_mean, np.float32)
    terminals_mean = np.asarray(terminals_mean, np.float32)
    actions = np.asarray(actions, np.float32)

    nets = {}
    for key, params in (("c", critic_params), ("t", target_params), ("a", actor_params)):
        nets[key] = _prep_net(params)

    shared = {"bdiag": np.kron(np.eye(4, dtype=np.float32),
                               np.ones((32, 32), np.float32)).astype(bf16)}
    for nk, (pre, Wf_aug) in nets.items():
        shared[f"wf_{nk}"] = np.ascontiguousarray(Wf_aug)
        for l, p in enumerate(pre):
            shared[f"w_{nk}_{l}"] = np.ascontiguousarray(p[0])
            shared[f"p_{nk}_{l}"] = _param_tile(p)

    in_maps = []
    for core in range(NCORES):
        msl = slice(core * MC, (core + 1) * MC)
        d = dict(shared)
        x0 = features[::-1, msl, :].transpose(2, 1, 0).reshape(F, T)
        d["x0"] = np.ascontiguousarray(x0.astype(bf16))
        d["rew"] = np.ascontiguousarray(rewards_mean[::-1, msl].T)
        d["term"] = np.ascontiguousarray(terminals_mean[::-1, msl].T)
        act_u = np.zeros((J, MC, A), np.float32)
        act_u[1:] = actions[::-1, msl, :]
        act_T = act_u.transpose(2, 1, 0).reshape(A, T)          # [32, 8192]
        act_stk = act_T.reshape(A, NG, NT, TW).transpose(1, 2, 0, 3).reshape(NG, 128, TW)
        d["act_stk"] = np.ascontiguousarray(act_stk.astype(bf16))
        in_maps.append(d)
    return in_maps


# ----------------------------------------------------------------------------
# bass graph
# ----------------------------------------------------------------------------

def build_graph():
    import contextlib
    import concourse.bass as bass
    import concourse.tile as tile
    from concourse import bacc, mybir

    f32 = mybir.dt.float32
    b16 = mybir.dt.bfloat16
    AF = mybir.ActivationFunctionType
    OP = mybir.AluOpType

    nc = bacc.Bacc()

    x0_e = nc.declare_dram_parameter("x0", [F, T], b16, isOutput=False)
    rew_e = nc.declare_dram_parameter("rew", [MC, J], f32, isOutput=False)
    term_e = nc.declare_dram_parameter("term", [MC, J], f32, isOutput=False)
    act_e = nc.declare_dram_parameter("act_stk", [NG, 128, TW], b16, isOutput=False)
    bdiag_e = nc.declare_dram_parameter("bdiag", [128, 128], b16, isOutput=False)
    w_e, p_e, wf_e = {}, {}, {}
    for nk, dout in _NETS:
        wf_e[nk] = nc.declare_dram_parameter(f"wf_{nk}", [HID + 1, dout], b16, isOutput=False)
        for l in range(NLAYERS):
            din = F if l == 0 else HID
            w_e[(nk, l)] = nc.declare_dram_parameter(f"w_{nk}_{l}", [din, HID], b16, isOutput=False)
            p_e[(nk, l)] = nc.declare_dram_parameter(f"p_{nk}_{l}", [128, 12], f32, isOutput=False)
    out_e = nc.declare_dram_parameter("out", [128, 8], f32, isOutput=True)

    vd = {nk: nc.dram_tensor(f"vd_{nk}", [NG * NT, TW], f32) for nk in ("c", "t")}
    gbuf = nc.dram_tensor("gbuf", [T], f32)
    rwbuf = nc.dram_tensor("rwbuf", [T], f32)

    with tile.TileContext(nc) as tc, contextlib.ExitStack() as ctx:
        persist = ctx.enter_context(tc.tile_pool(name="persist", bufs=1))
        wpool = ctx.enter_context(tc.tile_pool(name="wpool", bufs=1))
        xpool = ctx.enter_context(tc.tile_pool(name="xpool", bufs=2))
        x0pool = ctx.enter_context(tc.tile_pool(name="x0pool", bufs=1))
        mid = ctx.enter_context(tc.tile_pool(name="mid", bufs=1))
        ctp = ctx.enter_context(tc.tile_pool(name="ctp", bufs=1))
        small = ctx.enter_context(tc.tile_pool(name="small", bufs=2))
        headp = ctx.enter_context(tc.tile_pool(name="headp", bufs=1))
        psum_c = ctx.enter_context(tc.tile_pool(name="psum_c", bufs=5, space="PSUM"))
        psum_s = ctx.enter_context(tc.tile_pool(name="psum_s", bufs=3, space="PSUM"))

        # --- constants ---
        ones_kk = persist.tile([128, 128], b16, tag="ones_kk")
        nc.vector.memset(ones_kk, 1.0)
        bdiag = persist.tile([128, 128], b16, tag="bdiag")
        nc.sync.dma_start(out=bdiag, in_=bdiag_e[:, :])
        rew = persist.tile([MC, J], f32, tag="rew")
        nc.sync.dma_start(out=rew, in_=rew_e[:, :])
        term = persist.tile([MC, J], f32, tag="term")
        nc.sync.dma_start(out=term, in_=term_e[:, :])
        pol_cols = persist.tile([128, NG], f32, tag="pol_cols")
        ent_cols = persist.tile([128, NG], f32, tag="ent_cols")

        ptiles = {}
        for nk, _ in _NETS:
            for l in range(NLAYERS):
                pt = persist.tile([128, 12], f32, tag=f"p_{nk}_{l}")
                nc.sync.dma_start(out=pt, in_=p_e[(nk, l)][:, :])
                pt2 = persist.tile([128, 12], f32, name=f"p2_{nk}_{l}",
                                   tag=f"p2_{nk}_{l}")
                nc.scalar.copy(pt2, pt)
                ptiles[(nk, l)] = pt2

        def load_weights(nk):
            """DMA one net's weights into SBUF chunk tiles."""
            ws = {}
            for l in range(NLAYERS):
                ksz = KSZ_L0 if l == 0 else KSZ_H
                kofs = 0
                for ki, kn in enumerate(ksz):
                    wt = wpool.tile([kn, HID], b16, tag=f"w_l{l}_k{ki}")
                    nc.sync.dma_start(out=wt, in_=w_e[(nk, l)][kofs:kofs + kn, :])
                    ws[(l, ki)] = wt
                    kofs += kn
            dout = dict(_NETS)[nk]
            kofs = 0
            for ki, kn in enumerate(KSZ_F):
                wt = wpool.tile([kn, dout], b16, tag=f"wf_k{ki}")
                nc.sync.dma_start(out=wt, in_=wf_e[nk][kofs:kofs + kn, :])
                ws[("f", ki)] = wt
                kofs += kn
            return ws

        def stream_x0(g):
            gofs = g * GW
            xin = []
            for ki, kn in enumerate(KSZ_L0):
                xt = x0pool.tile([kn, GW], b16, name=f"x0_{ki}_{g % 2}",
                                 tag=f"x0_{ki}_{g % 2}")
                nc.sync.dma_start(out=xt, in_=x0_e[ki * 128:ki * 128 + kn, gofs:gofs + GW])
                xin.append(xt)
            return xin

        def emit_layer_pair(nk, ws, l, streams):
            """Emit one layer for two token groups, interleaved at tile
            granularity with 1-tile-delayed stats matmuls so PE never waits
            on the cast->square chain."""
            ksz = KSZ_L0 if l == 0 else KSZ_H
            pt = ptiles[(nk, l)]
            st = {}
            for g, cur in streams.items():
                s = g % 2
                st[g] = dict(
                    s=s, cur=cur,
                    ct=[ctp.tile([CSZ[c], GW], b16, name=f"ct{c}_{s}",
                                 tag=f"ct{c}_{s}") for c in range(4)],
                    lnv=mid.tile([128, GW], b16, name=f"lnv_{s}", tag=f"lnv_{s}"),
                    sqs={},
                )

            def mm_tile(g, t):
                d = st[g]
                tsl = slice(t * TW, (t + 1) * TW)
                for c in range(4):
                    csz = CSZ[c]
                    pc = psum_c.tile([csz, TW], f32, name="pc", tag="pc")
                    for ki, kn in enumerate(ksz):
                        nc.tensor.matmul(
                            pc, lhsT=ws[(l, ki)][:, COFS[c]:COFS[c] + csz],
                            rhs=d["cur"][ki][0:kn, tsl],
                            start=(ki == 0), stop=(ki == len(ksz) - 1))
                    dst = d["ct"][c][:, tsl]
                    if c < 3:
                        nc.scalar.activation(dst, pc, AF.Identity,
                                             bias=pt[0:csz, 3 * c:3 * c + 1],
                                             scale=1.0)
                    else:
                        nc.vector.tensor_scalar(dst, pc,
                                                pt[0:csz, 3 * c:3 * c + 1],
                                                None, OP.add)
                    sqt = small.tile([csz, TW], b16, name=f"sq{c}_{d['s']}",
                                     tag=f"sq{c}_{d['s']}")
                    # chunk 3 is 16 rows; gpsimd cost scales with free size
                    # only, so square it on DVE (2x bf16) instead
                    if c == 3:
                        nc.vector.tensor_mul(sqt, dst, dst)
                    else:
                        nc.gpsimd.tensor_mul(sqt, dst, dst)
                    d["sqs"][(t, c)] = sqt

            def stats_tile(g, t):
                d = st[g]
                tsl = slice(t * TW, (t + 1) * TW)
                ps = psum_s.tile([128, TW], f32, name="ps", tag="ps")
                for c in range(4):
                    nc.tensor.matmul(ps, lhsT=ones_kk[0:CSZ[c], :],
                                     rhs=d["sqs"].pop((t, c)),
                                     start=(c == 0), stop=(c == 3))
                nc.scalar.activation(d["lnv"][:, tsl], ps, AF.Ln,
                                     scale=1.0 / (HID * LN_EPS), bias=1.0)

            gs = list(streams)
            pending = []
            for t in range(NT):
                for g in gs:
                    mm_tile(g, t)
                    pending.append((g, t))
                    if len(pending) > 2:
                        stats_tile(*pending.pop(0))
            for item in pending:
                stats_tile(*item)

            out = {}
            for g in gs:
                d = st[g]
                s = d["s"]
                # rg = (var/eps + 1)^-0.5 = rstd*sqrt(eps); eps^-0.5 and g
                # factors are folded into the v-scale param column
                rg = mid.tile([128, GW], b16, name=f"rg_{s}", tag=f"rg_{s}")
                nc.scalar.activation(rg, d["lnv"], AF.Exp, scale=-0.5)
                nxt = []
                for c in range(4):
                    csz = CSZ[c]
                    uu = mid.tile([csz, GW], b16, name=f"uu_{s}", tag=f"uu_{s}")
                    nc.vector.tensor_mul(uu, d["ct"][c], rg[0:csz, :])
                    vv = mid.tile([csz, GW], b16, name=f"vv_{s}", tag=f"vv_{s}")
                    nc.vector.tensor_scalar(vv, uu, pt[0:csz, 3 * c + 1:3 * c + 2],
                                            pt[0:csz, 3 * c + 2:3 * c + 3],
                                            OP.mult, OP.add)
                    ee = mid.tile([csz, GW], b16, name=f"ee_{s}", tag=f"lnv_{s}")
                    nc.scalar.activation(ee, vv, AF.Exp)
                    # elu(v) = max(v, min(e^v, 1) - 1)  (exp-first, overflow-safe)
                    tt = mid.tile([csz, GW], b16, name=f"tt_{s}", tag=f"uu_{s}")
                    nc.vector.tensor_scalar(tt, ee, 1.0, 1.0, OP.min, OP.subtract)
                    xc = xpool.tile([csz + (1 if c == 3 else 0), GW], b16,
                                    name=f"xn{c}_{s}", tag=f"xn{c}_{s}")
                    if c == 3 and l == NLAYERS - 1:
                        # row 16 doubles as the ones row feeding the head's
                        # bias matmul chunk; partition-16 writes are illegal
                        # so fill the whole tile first.
                        nc.gpsimd.memset(xc, 1.0)
                    nc.vector.tensor_max(xc[0:csz, :], vv, tt)
                    nxt.append(xc)
                out[g] = nxt
            return out

        def value_head(nk, ws, g, x4):
            for t in range(NT):
                tsl = slice(t * TW, (t + 1) * TW)
                pv = psum_c.tile([1, TW], f32, tag="pc")
                for ki, kn in enumerate(KSZ_F):
                    nc.tensor.matmul(pv, lhsT=ws[("f", ki)], rhs=x4[ki][0:kn, tsl],
                                     start=(ki == 0), stop=(ki == 3))
                vtmp = small.tile([1, TW], f32, tag="vtmp")
                nc.scalar.copy(vtmp, pv)
                nc.sync.dma_start(out=vd[nk][g * NT + t:g * NT + t + 1, :], in_=vtmp)

        # ---- critic + target nets (two token groups interleaved so PE
        # works on one group's matmuls during the other's elementwise tail) ----
        for nk in ("c", "t"):
            ws = load_weights(nk)
            for g0 in (0, 2):
                curs = {g: stream_x0(g) for g in (g0, g0 + 1)}
                for l in range(NLAYERS):
                    curs = emit_layer_pair(nk, ws, l, curs)
                for g in (g0, g0 + 1):
                    value_head(nk, ws, g, curs[g])

        # ---- GAE / reality weight / value loss ----
        import concourse.bass as bass_mod
        v_sb = persist.tile([MC, J], f32, tag="v_sb")
        vt_sb = persist.tile([MC, J], f32, tag="vt_sb")
        for nk, dst in (("c", v_sb), ("t", vt_sb)):
            src = bass_mod.AP(tensor=vd[nk][:, :].tensor, offset=0,
                              ap=[[J, MC], [1, J]])
            nc.sync.dma_start(out=dst, in_=src)

        gae = persist
        q = gae.tile([MC, H], f32, tag="q")
        nc.vector.tensor_scalar(q, term[:, 0:H], -GAMMA, GAMMA, OP.mult, OP.add)
        s1 = gae.tile([MC, H], f32, tag="s1")
        nc.vector.tensor_mul(s1, q, vt_sb[:, 0:H])
        s2 = gae.tile([MC, H], f32, tag="s2")
        nc.vector.tensor_sub(s2, rew[:, 0:H], vt_sb[:, 1:J])
        adv = gae.tile([MC, H], f32, tag="adv")
        nc.vector.tensor_add(adv, s1, s2)
        cl = gae.tile([MC, H], f32, tag="cl")
        nc.vector.tensor_scalar(cl, q, LAMBDA, None, OP.mult)
        agae = gae.tile([MC, H], f32, tag="agae")
        nc.vector.tensor_tensor_scan(agae, cl, adv, 0.0, OP.mult, OP.add)
        vtar = gae.tile([MC, H], f32, tag="vtar")
        nc.vector.tensor_add(vtar, agae, vt_sb[:, 1:J])
        delta = gae.tile([MC, H], f32, tag="delta")
        nc.vector.tensor_sub(delta, vtar, v_sb[:, 1:J])

        lg = gae.tile([MC, H], f32, tag="lg")
        nc.scalar.activation(lg, term[:, 1:J], AF.Ln, scale=-1.0, bias=1.0)
        S = gae.tile([MC, 1], f32, tag="S")
        nc.vector.tensor_reduce(S, lg, axis=mybir.AxisListType.X, op=OP.add)
        ones_mc = gae.tile([MC, H], f32, tag="ones_mc")
        nc.vector.memset(ones_mc, 1.0)
        cinc = gae.tile([MC, H], f32, tag="cinc")
        nc.vector.tensor_tensor_scan(cinc, ones_mc, lg, 0.0, OP.mult, OP.add)
        cx = gae.tile([MC, H], f32, tag="cx")
        nc.vector.tensor_sub(cx, cinc, lg)
        rwf = gae.tile([MC, J], f32, tag="rwf")
        nc.vector.memset(rwf[:, 0:1], 0.0)
        nc.scalar.activation(rwf[:, 1:J], cx, AF.Exp, scale=-1.0, bias=S)
        gfull = gae.tile([MC, J], f32, tag="gfull")
        nc.vector.memset(gfull[:, 0:1], 0.0)
        nc.vector.tensor_mul(gfull[:, 1:J], agae, rwf[:, 1:J])

        d2 = gae.tile([MC, H], f32, tag="d2")
        nc.vector.tensor_mul(d2, delta, delta)
        val_acc = gae.tile([MC, 1], f32, tag="val_acc")
        scrap64 = gae.tile([MC, H], f32, tag="scrap64")
        nc.vector.scalar_tensor_tensor(scrap64, d2, 1.0, rwf[:, 1:J],
                                       OP.mult, OP.mult, accum_out=val_acc)
        nc.sync.dma_start(out=out_e[0:MC, 0:1], in_=val_acc)

        gdst = bass_mod.AP(tensor=gbuf[:].tensor, offset=0, ap=[[J, MC], [1, J]])
        rwdst = bass_mod.AP(tensor=rwbuf[:].tensor, offset=0, ap=[[J, MC], [1, J]])
        nc.sync.dma_start(out=gdst, in_=gfull)
        nc.sync.dma_start(out=rwdst, in_=rwf)

        # ---- actor net + policy head ----
        ws = load_weights("a")
        x4s = {}
        for g0 in (0, 2):
            curs = {g: stream_x0(g) for g in (g0, g0 + 1)}
            for l in range(NLAYERS):
                curs = emit_layer_pair("a", ws, l, curs)
            x4s.update(curs)
        for g in range(NG):
            x4 = x4s[g]
            pz = psum_s.tile([128, TW], f32, tag="ps")
            for b in range(4):
                bsl = slice(b * TW, (b + 1) * TW)
                for ki, kn in enumerate(KSZ_F):
                    nc.tensor.matmul(pz[32 * b:32 * b + 32, :], lhsT=ws[("f", ki)],
                                     rhs=x4[ki][0:kn, bsl],
                                     start=(ki == 0), stop=(ki == 3),
                                     tile_position=(0, 32 * b))
            a_sb = headp.tile([128, TW], b16, tag="a_sb")
            nc.sync.dma_start(out=a_sb, in_=act_e[g])
            E = headp.tile([128, TW], b16, tag="E")
            nc.scalar.activation(E, pz, AF.Exp)
            Ez = headp.tile([128, TW], b16, tag="Ez")
            nc.vector.tensor_mul(Ez, pz, E)
            az = headp.tile([128, TW], b16, tag="az")
            nc.vector.tensor_mul(az, pz, a_sb)
            sums = {}
            for qn, src in (("S", E), ("W", Ez), ("U", az), ("B", a_sb)):
                p = psum_s.tile([128, TW], f32, tag="ps")
                nc.tensor.matmul(p, lhsT=bdiag, rhs=src, start=True, stop=True)
                sums[qn] = p
            lnS = headp.tile([128, TW], f32, tag="lnS")
            nc.scalar.activation(lnS, sums["S"], AF.Ln)
            Sinv = headp.tile([128, TW], b16, tag="Sinv")
            nc.scalar.activation(Sinv, lnS, AF.Exp, scale=-1.0)
            t1 = headp.tile([128, TW], b16, tag="t1")
            nc.vector.tensor_mul(t1, sums["B"], lnS)
            alp = headp.tile([128, TW], b16, tag="alp")
            nc.vector.tensor_sub(alp, sums["U"], t1)
            t2 = headp.tile([128, TW], b16, tag="t2")
            nc.vector.tensor_mul(t2, sums["W"], Sinv)
            ent = headp.tile([128, TW], b16, tag="ent")
            nc.vector.tensor_sub(ent, lnS, t2)

            gstk = headp.tile([128, TW], f32, tag="gstk")
            gsrc = bass_mod.AP(tensor=gbuf[:].tensor, offset=g * GW,
                               ap=[[TW, 4], [0, 32], [1, TW]])
            nc.gpsimd.dma_start(out=gstk, in_=gsrc)
            rstk = headp.tile([128, TW], f32, tag="rstk")
            rsrc = bass_mod.AP(tensor=rwbuf[:].tensor, offset=g * GW,
                               ap=[[TW, 4], [0, 32], [1, TW]])
            nc.gpsimd.dma_start(out=rstk, in_=rsrc)
            scrap = headp.tile([128, TW], b16, tag="scrap")
            nc.vector.scalar_tensor_tensor(scrap, alp, 1.0, gstk, OP.mult, OP.mult,
                                           accum_out=pol_cols[:, g:g + 1])
            scrap2 = headp.tile([128, TW], b16, tag="scrap2")
            nc.vector.scalar_tensor_tensor(scrap2, ent, 1.0, rstk, OP.mult, OP.mult,
                                           accum_out=ent_cols[:, g:g + 1])

        pol_sum = persist.tile([128, 1], f32, tag="pol_sum")
        nc.vector.tensor_reduce(pol_sum, pol_cols, axis=mybir.AxisListType.X, op=OP.add)
        ent_sum = persist.tile([128, 1], f32, tag="ent_sum")
        nc.vector.tensor_reduce(ent_sum, ent_cols, axis=mybir.AxisListType.X, op=OP.add)
        nc.sync.dma_start(out=out_e[:, 1:2], in_=pol_sum)
        nc.sync.dma_start(out=out_e[:, 2:3], in_=ent_sum)

    import concourse.bacc as bacc_mod
    import concourse.hw_specs as hw_specs
    orig_tables = hw_specs.get_activation_tables
    keep = "natural_log_exp_and_others"
    mine = {AF.Identity, AF.Copy, AF.Exp, AF.Ln, AF.Square}

    def patched_tables(arch):
        out = {}
        for name, s in orig_tables(arch).items():
            out[name] = s if name == keep else (s - mine)
        return out

    bacc_mod.get_activation_tables = patched_tables
    try:
        nc.compile()
    finally:
        bacc_mod.get_activation_tables = orig_tables
    return nc


_NC_CACHE = {}


def kernel(features, rewards_mean, terminals_mean, actions,
           actor_params, critic_params, target_params, _want_timing=False):
    from concourse.bass_utils import run_bass_kernel_spmd

    in_maps = prep_all(features, rewards_mean, terminals_mean, actions,
                       actor_params, critic_params, target_params)
    if "nc" not in _NC_CACHE:
        _NC_CACHE["nc"] = build_graph()
    nc = _NC_CACHE["nc"]
    res = run_bass_kernel_spmd(nc, in_maps, core_ids=list(range(NCORES)),
                               trace=_want_timing)
    outs = [np.asarray(r["out"], np.float64) for r in res.results]
    val = sum(o[0:MC, 0].sum() for o in outs)
    pol = sum(o[:, 1].sum() for o in outs) / 32.0
    ent = sum(o[:, 2].sum() for o in outs) / 32.0
    denom = float(H * M)
    loss_value = 0.5 * val / denom
    loss_policy = -pol / denom
    policy_entropy = ent / denom
    loss_actor = loss_policy - TEMPERATURE * policy_entropy
    out = np.array([loss_actor, loss_value], np.float32)
    if _want_timing:
        return out, res
    return out


# revision 28
# speedup vs baseline: 1.4081x; 1.0228x over previous
"""Trainium2 Bass kernel for nn_ActorCritic loss (8-core SPMD, data-parallel over batch).

Strategy:
- Shard batch M=512 over 8 cores (64 each). MLP params replicated.
- Host prep: features pre-transposed to feature-major [F, T] per core with token
  column t = m*J + u, u = J-1-j (time-reversed so the GAE recursion becomes a
  forward hardware scan). LayerNorm mean-subtraction is folded into the weights
  (W - rowmean, b - mean); LN gain g is folded into the rstd computation via
  exp(-0.5*ln(var+eps) + ln g).
- On chip per core: 3 MLPs (critic/target/actor) in feature-major layout, bf16
  matmuls with fp32 PSUM. Per-token variance via ones-matrix matmul broadcast.
  ELU via exp/min/max with scalar_tensor_tensor fusion. GAE via
  tensor_tensor_scan. Reality weight via suffix-cumsum trick (no reversal).
  Actor head: stacked [4x32, 512] logits tiles, block-diagonal ones matmul for
  per-token reductions over A.
- Each core emits partial loss sums; host combines to the final [2] output.
"""
import os
import sys
import numpy as np

for _p in ("/opt/trn_rl_repo",):
    if _p not in sys.path and os.path.isdir(_p):
        sys.path.insert(0, _p)

import ml_dtypes  # noqa: E402

bf16 = ml_dtypes.bfloat16

GAMMA = 0.999
LAMBDA = 0.95
TEMPERATURE = 0.001
J, M, F, A = 128, 512, 256, 32
H = J - 1
HID, NLAYERS = 400, 4
LN_EPS = 1e-3
NCORES = 8
MC = M // NCORES          # 64 batch elements per core
T = MC * J                # 8192 tokens per core
NG = 4                    # token groups
GW = T // NG              # 2048 cols per group
NT = 4                    # 512-col tiles per group
TW = GW // NT             # 512

CSZ = [128, 128, 128, 16]         # dout chunks of HID=400
COFS = [0, 128, 256, 384]
KSZ_L0 = [128, 128]               # din chunks of F=256
KSZ_H = [128, 128, 128, 16]       # din chunks of HID=400
KSZ_F = [128, 128, 128, 17]       # din chunks of HID+1 (bias row)

_NETS = [("c", 1), ("t", 1), ("a", 32)]


# ----------------------------------------------------------------------------
# host-side prep
# ----------------------------------------------------------------------------

def _prep_net(params):
    hidden, (Wf, bf_) = params
    pre = []
    for (W, b, g, beta) in hidden:
        W = np.asarray(W, np.float32)
        b = np.asarray(b, np.float32)
        g = np.asarray(g, np.float32)
        beta = np.asarray(beta, np.float32)
        Wc = W - W.mean(axis=1, keepdims=True)
        bc = b - b.mean()
        pre.append((Wc.astype(bf16), bc, g, beta))
    Wf = np.asarray(Wf, np.float32)
    bf_ = np.asarray(bf_, np.float32)
    Wf_aug = np.concatenate([Wf, bf_[None, :]], axis=0).astype(bf16)
    return pre, Wf_aug


def _param_tile(pre_l):
    """[128, 12] fp32: col 3c+0 = centered bias, 3c+1 = ln g, 3c+2 = beta for chunk c."""
    _, bc, g, beta = pre_l
    out = np.zeros((128, 12), np.float32)
    for c in range(4):
        n = CSZ[c]
        sl = slice(COFS[c], COFS[c] + n)
        out[:n, 3 * c + 0] = bc[sl]
        out[:n, 3 * c + 1] = g[sl] * LN_EPS ** -0.5
        out[:n, 3 * c + 2] = beta[sl]
    return out


def prep_all(features, rewards_mean, terminals_mean, actions,
             actor_params, critic_params, target_params):
    features = np.asarray(features, np.float32)
    rewards_mean = np.asarray(rewards_mean, np.float32)
    terminals_mean = np.asarray(terminals_mean, np.float32)
    actions = np.asarray(actions, np.float32)

    nets = {}
    for key, params in (("c", critic_params), ("t", target_params), ("a", actor_params)):
        nets[key] = _prep_net(params)

    shared = {"bdiag": np.kron(np.eye(4, dtype=np.float32),
                               np.ones((32, 32), np.float32)).astype(bf16)}
    for nk, (pre, Wf_aug) in nets.items():
        shared[f"wf_{nk}"] = np.ascontiguousarray(Wf_aug)
        for l, p in enumerate(pre):
            shared[f"w_{nk}_{l}"] = np.ascontiguousarray(p[0])
            shared[f"p_{nk}_{l}"] = _param_tile(p)

    in_maps = []
    for core in range(NCORES):
        msl = slice(core * MC, (core + 1) * MC)
        d = dict(shared)
        x0 = features[::-1, msl, :].transpose(2, 1, 0).reshape(F, T)
        d["x0"] = np.ascontiguousarray(x0.astype(bf16))
        d["rew"] = np.ascontiguousarray(rewards_mean[::-1, msl].T)
        d["term"] = np.ascontiguousarray(terminals_mean[::-1, msl].T)
        act_u = np.zeros((J, MC, A), np.float32)
        act_u[1:] = actions[::-1, msl, :]
        act_T = act_u.transpose(2, 1, 0).reshape(A, T)          # [32, 8192]
        act_stk = act_T.reshape(A, NG, NT, TW).transpose(1, 2, 0, 3).reshape(NG, 128, TW)
        d["act_stk"] = np.ascontiguousarray(act_stk.astype(bf16))
        in_maps.append(d)
    return in_maps


# ----------------------------------------------------------------------------
# bass graph
# ----------------------------------------------------------------------------

def build_graph():
    import contextlib
    import concourse.bass as bass
    import concourse.tile as tile
    from concourse import bacc, mybir

    f32 = mybir.dt.float32
    b16 = mybir.dt.bfloat16
    AF = mybir.ActivationFunctionType
    OP = mybir.AluOpType

    nc = bacc.Bacc()

    x0_e = nc.declare_dram_parameter("x0", [F, T], b16, isOutput=False)
    rew_e = nc.declare_dram_parameter("rew", [MC, J], f32, isOutput=False)
    term_e = nc.declare_dram_parameter("term", [MC, J], f32, isOutput=False)
    act_e = nc.declare_dram_parameter("act_stk", [NG, 128, TW], b16, isOutput=False)
    bdiag_e = nc.declare_dram_parameter("bdiag", [128, 128], b16, isOutput=False)
    w_e, p_e, wf_e = {}, {}, {}
    for nk, dout in _NETS:
        wf_e[nk] = nc.declare_dram_parameter(f"wf_{nk}", [HID + 1, dout], b16, isOutput=False)
        for l in range(NLAYERS):
            din = F if l == 0 else HID
            w_e[(nk, l)] = nc.declare_dram_parameter(f"w_{nk}_{l}", [din, HID], b16, isOutput=False)
            p_e[(nk, l)] = nc.declare_dram_parameter(f"p_{nk}_{l}", [128, 12], f32, isOutput=False)
    out_e = nc.declare_dram_parameter("out", [128, 8], f32, isOutput=True)

    vd = {nk: nc.dram_tensor(f"vd_{nk}", [NG * NT, TW], f32) for nk in ("c", "t")}
    gbuf = nc.dram_tensor("gbuf", [T], f32)
    rwbuf = nc.dram_tensor("rwbuf", [T], f32)

    with tile.TileContext(nc) as tc, contextlib.ExitStack() as ctx:
        persist = ctx.enter_context(tc.tile_pool(name="persist", bufs=1))
        wpool = ctx.enter_context(tc.tile_pool(name="wpool", bufs=1))
        xpool = ctx.enter_context(tc.tile_pool(name="xpool", bufs=2))
        x0pool = ctx.enter_context(tc.tile_pool(name="x0pool", bufs=1))
        mid = ctx.enter_context(tc.tile_pool(name="mid", bufs=1))
        ctp = ctx.enter_context(tc.tile_pool(name="ctp", bufs=1))
        small = ctx.enter_context(tc.tile_pool(name="small", bufs=2))
        headp = ctx.enter_context(tc.tile_pool(name="headp", bufs=1))
        psum_c = ctx.enter_context(tc.tile_pool(name="psum_c", bufs=5, space="PSUM"))
        psum_s = ctx.enter_context(tc.tile_pool(name="psum_s", bufs=3, space="PSUM"))

        # --- constants ---
        ones_kk = persist.tile([128, 128], b16, tag="ones_kk")
        nc.vector.memset(ones_kk, 1.0)
        bdiag = persist.tile([128, 128], b16, tag="bdiag")
        nc.sync.dma_start(out=bdiag, in_=bdiag_e[:, :])
        rew = persist.tile([MC, J], f32, tag="rew")
        nc.sync.dma_start(out=rew, in_=rew_e[:, :])
        term = persist.tile([MC, J], f32, tag="term")
        nc.sync.dma_start(out=term, in_=term_e[:, :])
        pol_cols = persist.tile([128, NG], f32, tag="pol_cols")
        ent_cols = persist.tile([128, NG], f32, tag="ent_cols")

        ptiles = {}
        for nk, _ in _NETS:
            for l in range(NLAYERS):
                pt = persist.tile([128, 12], f32, tag=f"p_{nk}_{l}")
                nc.sync.dma_start(out=pt, in_=p_e[(nk, l)][:, :])
                pt2 = persist.tile([128, 12], f32, name=f"p2_{nk}_{l}",
                                   tag=f"p2_{nk}_{l}")
                nc.scalar.copy(pt2, pt)
                ptiles[(nk, l)] = pt2

        def load_weights(nk):
            """DMA one net's weights into SBUF chunk tiles."""
            ws = {}
            for l in range(NLAYERS):
                ksz = KSZ_L0 if l == 0 else KSZ_H
                kofs = 0
                for ki, kn in enumerate(ksz):
                    wt = wpool.tile([kn, HID], b16, tag=f"w_l{l}_k{ki}")
                    nc.sync.dma_start(out=wt, in_=w_e[(nk, l)][kofs:kofs + kn, :])
                    ws[(l, ki)] = wt
                    kofs += kn
            dout = dict(_NETS)[nk]
            kofs = 0
            for ki, kn in enumerate(KSZ_F):
                wt = wpool.tile([kn, dout], b16, tag=f"wf_k{ki}")
                nc.sync.dma_start(out=wt, in_=wf_e[nk][kofs:kofs + kn, :])
                ws[("f", ki)] = wt
                kofs += kn
            return ws

        def stream_x0(g):
            gofs = g * GW
            xin = []
            for ki, kn in enumerate(KSZ_L0):
                xt = x0pool.tile([kn, GW], b16, name=f"x0_{ki}_{g % 2}",
                                 tag=f"x0_{ki}_{g % 2}")
                nc.sync.dma_start(out=xt, in_=x0_e[ki * 128:ki * 128 + kn, gofs:gofs + GW])
                xin.append(xt)
            return xin

        def emit_layer_pair(nk, ws, l, streams):
            """Emit one layer for two token groups, interleaved at tile
            granularity with 1-tile-delayed stats matmuls so PE never waits
            on the cast->square chain."""
            ksz = KSZ_L0 if l == 0 else KSZ_H
            pt = ptiles[(nk, l)]
            st = {}
            for g, cur in streams.items():
                s = g % 2
                st[g] = dict(
                    s=s, cur=cur,
                    ct=[ctp.tile([CSZ[c], GW], b16, name=f"ct{c}_{s}",
                                 tag=f"ct{c}_{s}") for c in range(4)],
                    lnv=mid.tile([128, GW], b16, name=f"lnv_{s}", tag=f"lnv_{s}"),
                    sqs={},
                )

            def mm_tile(g, t):
                d = st[g]
                tsl = slice(t * TW, (t + 1) * TW)
                for c in range(4):
                    csz = CSZ[c]
                    pc = psum_c.tile([csz, TW], f32, name="pc", tag="pc")
                    for ki, kn in enumerate(ksz):
                        nc.tensor.matmul(
                            pc, lhsT=ws[(l, ki)][:, COFS[c]:COFS[c] + csz],
                            rhs=d["cur"][ki][0:kn, tsl],
                            start=(ki == 0), stop=(ki == len(ksz) - 1))
                    dst = d["ct"][c][:, tsl]
                    # PSUM exit split across ACT/DVE for engine balance
                    on_act = c < 2 or (c == 2 and d["s"] == 1)
                    if on_act:
                        nc.scalar.activation(dst, pc, AF.Identity,
                                             bias=pt[0:csz, 3 * c:3 * c + 1],
                                             scale=1.0)
                    else:
                        nc.vector.tensor_scalar(dst, pc,
                                                pt[0:csz, 3 * c:3 * c + 1],
                                                None, OP.add)
                    sqt = small.tile([csz, TW], b16, name=f"sq{c}_{d['s']}",
                                     tag=f"sq{c}_{d['s']}")
                    # chunk 3 is 16 rows; gpsimd cost scales with free size
                    # only, so square it on DVE (2x bf16) instead
                    if c == 3:
                        nc.vector.tensor_mul(sqt, dst, dst)
                    else:
                        nc.gpsimd.tensor_mul(sqt, dst, dst)
                    d["sqs"][(t, c)] = sqt

            def stats_tile(g, t):
                d = st[g]
                tsl = slice(t * TW, (t + 1) * TW)
                ps = psum_s.tile([128, TW], f32, name="ps", tag="ps")
                for c in range(4):
                    nc.tensor.matmul(ps, lhsT=ones_kk[0:CSZ[c], :],
                                     rhs=d["sqs"].pop((t, c)),
                                     start=(c == 0), stop=(c == 3))
                nc.scalar.activation(d["lnv"][:, tsl], ps, AF.Ln,
                                     scale=1.0 / (HID * LN_EPS), bias=1.0)

            gs = list(streams)
            pending = []
            for t in range(NT):
                for g in gs:
                    mm_tile(g, t)
                    pending.append((g, t))
                    if len(pending) > 2:
                        stats_tile(*pending.pop(0))
            for item in pending:
                stats_tile(*item)

            out = {}
            for g in gs:
                d = st[g]
                s = d["s"]
                # rg = (var/eps + 1)^-0.5 = rstd*sqrt(eps); eps^-0.5 and g
                # factors are folded into the v-scale param column
                rg = mid.tile([128, GW], b16, name=f"rg_{s}", tag=f"rg_{s}")
                nc.scalar.activation(rg, d["lnv"], AF.Exp, scale=-0.5)
                nxt = []
                for c in range(4):
                    csz = CSZ[c]
                    uu = mid.tile([csz, GW], b16, name=f"uu_{s}", tag=f"uu_{s}")
                    nc.vector.tensor_mul(uu, d["ct"][c], rg[0:csz, :])
                    vv = mid.tile([csz, GW], b16, name=f"vv_{s}", tag=f"vv_{s}")
                    nc.vector.tensor_scalar(vv, uu, pt[0:csz, 3 * c + 1:3 * c + 2],
                                            pt[0:csz, 3 * c + 2:3 * c + 3],
                                            OP.mult, OP.add)
                    ee = mid.tile([csz, GW], b16, name=f"ee_{s}", tag=f"lnv_{s}")
                    nc.scalar.activation(ee, vv, AF.Exp)
                    # elu(v) = max(v, min(e^v, 1) - 1)  (exp-first, overflow-safe)
                    tt = mid.tile([csz, GW], b16, name=f"tt_{s}", tag=f"uu_{s}")
                    nc.vector.tensor_scalar(tt, ee, 1.0, 1.0, OP.min, OP.subtract)
                    xc = xpool.tile([csz + (1 if c == 3 else 0), GW], b16,
                                    name=f"xn{c}_{s}", tag=f"xn{c}_{s}")
                    if c == 3 and l == NLAYERS - 1:
                        # row 16 doubles as the ones row feeding the head's
                        # bias matmul chunk; partition-16 writes are illegal
                        # so fill the whole tile first.
                        nc.gpsimd.memset(xc, 1.0)
                    nc.vector.tensor_max(xc[0:csz, :], vv, tt)
                    nxt.append(xc)
                out[g] = nxt
            return out

        def value_head(nk, ws, g, x4):
            for t in range(NT):
                tsl = slice(t * TW, (t + 1) * TW)
                pv = psum_c.tile([1, TW], f32, tag="pc")
                for ki, kn in enumerate(KSZ_F):
                    nc.tensor.matmul(pv, lhsT=ws[("f", ki)], rhs=x4[ki][0:kn, tsl],
                                     start=(ki == 0), stop=(ki == 3))
                vtmp = small.tile([1, TW], f32, tag="vtmp")
                nc.scalar.copy(vtmp, pv)
                nc.sync.dma_start(out=vd[nk][g * NT + t:g * NT + t + 1, :], in_=vtmp)

        # ---- critic + target nets (two token groups interleaved so PE
        # works on one group's matmuls during the other's elementwise tail) ----
        for nk in ("c", "t"):
            ws = load_weights(nk)
            for g0 in (0, 2):
                curs = {g: stream_x0(g) for g in (g0, g0 + 1)}
                for l in range(NLAYERS):
                    curs = emit_layer_pair(nk, ws, l, curs)
                for g in (g0, g0 + 1):
                    value_head(nk, ws, g, curs[g])

        # ---- GAE / reality weight / value loss ----
        import concourse.bass as bass_mod
        v_sb = persist.tile([MC, J], f32, tag="v_sb")
        vt_sb = persist.tile([MC, J], f32, tag="vt_sb")
        for nk, dst in (("c", v_sb), ("t", vt_sb)):
            src = bass_mod.AP(tensor=vd[nk][:, :].tensor, offset=0,
                              ap=[[J, MC], [1, J]])
            nc.sync.dma_start(out=dst, in_=src)

        gae = persist
        q = gae.tile([MC, H], f32, tag="q")
        nc.vector.tensor_scalar(q, term[:, 0:H], -GAMMA, GAMMA, OP.mult, OP.add)
        s1 = gae.tile([MC, H], f32, tag="s1")
        nc.vector.tensor_mul(s1, q, vt_sb[:, 0:H])
        s2 = gae.tile([MC, H], f32, tag="s2")
        nc.vector.tensor_sub(s2, rew[:, 0:H], vt_sb[:, 1:J])
        adv = gae.tile([MC, H], f32, tag="adv")
        nc.vector.tensor_add(adv, s1, s2)
        cl = gae.tile([MC, H], f32, tag="cl")
        nc.vector.tensor_scalar(cl, q, LAMBDA, None, OP.mult)
        agae = gae.tile([MC, H], f32, tag="agae")
        nc.vector.tensor_tensor_scan(agae, cl, adv, 0.0, OP.mult, OP.add)
        vtar = gae.tile([MC, H], f32, tag="vtar")
        nc.vector.tensor_add(vtar, agae, vt_sb[:, 1:J])
        delta = gae.tile([MC, H], f32, tag="delta")
        nc.vector.tensor_sub(delta, vtar, v_sb[:, 1:J])

        lg = gae.tile([MC, H], f32, tag="lg")
        nc.scalar.activation(lg, term[:, 1:J], AF.Ln, scale=-1.0, bias=1.0)
        S = gae.tile([MC, 1], f32, tag="S")
        nc.vector.tensor_reduce(S, lg, axis=mybir.AxisListType.X, op=OP.add)
        ones_mc = gae.tile([MC, H], f32, tag="ones_mc")
        nc.vector.memset(ones_mc, 1.0)
        cinc = gae.tile([MC, H], f32, tag="cinc")
        nc.vector.tensor_tensor_scan(cinc, ones_mc, lg, 0.0, OP.mult, OP.add)
        cx = gae.tile([MC, H], f32, tag="cx")
        nc.vector.tensor_sub(cx, cinc, lg)
        rwf = gae.tile([MC, J], f32, tag="rwf")
        nc.vector.memset(rwf[:, 0:1], 0.0)
        nc.scalar.activation(rwf[:, 1:J], cx, AF.Exp, scale=-1.0, bias=S)
        gfull = gae.tile([MC, J], f32, tag="gfull")
        nc.vector.memset(gfull[:, 0:1], 0.0)
        nc.vector.tensor_mul(gfull[:, 1:J], agae, rwf[:, 1:J])

        d2 = gae.tile([MC, H], f32, tag="d2")
        nc.vector.tensor_mul(d2, delta, delta)
        val_acc = gae.tile([MC, 1], f32, tag="val_acc")
        scrap64 = gae.tile([MC, H], f32, tag="scrap64")
        nc.vector.scalar_tensor_tensor(scrap64, d2, 1.0, rwf[:, 1:J],
                                       OP.mult, OP.mult, accum_out=val_acc)
        nc.sync.dma_start(out=out_e[0:MC, 0:1], in_=val_acc)

        gdst = bass_mod.AP(tensor=gbuf[:].tensor, offset=0, ap=[[J, MC], [1, J]])
        rwdst = bass_mod.AP(tensor=rwbuf[:].tensor, offset=0, ap=[[J, MC], [1, J]])
        nc.sync.dma_start(out=gdst, in_=gfull)
        nc.sync.dma_start(out=rwdst, in_=rwf)

        # ---- actor net + policy head ----
        ws = load_weights("a")
        x4s = {}
        for g0 in (0, 2):
            curs = {g: stream_x0(g) for g in (g0, g0 + 1)}
            for l in range(NLAYERS):
                curs = emit_layer_pair("a", ws, l, curs)
            x4s.update(curs)
        for g in range(NG):
            x4 = x4s[g]
            pz = psum_s.tile([128, TW], f32, tag="ps")
            for b in range(4):
                bsl = slice(b * TW, (b + 1) * TW)
                for ki, kn in enumerate(KSZ_F):
                    nc.tensor.matmul(pz[32 * b:32 * b + 32, :], lhsT=ws[("f", ki)],
                                     rhs=x4[ki][0:kn, bsl],
                                     start=(ki == 0), stop=(ki == 3),
                                     tile_position=(0, 32 * b))
            a_sb = headp.tile([128, TW], b16, tag="a_sb")
            nc.sync.dma_start(out=a_sb, in_=act_e[g])
            E = headp.tile([128, TW], b16, tag="E")
            nc.scalar.activation(E, pz, AF.Exp)
            Ez = headp.tile([128, TW], b16, tag="Ez")
            nc.vector.tensor_mul(Ez, pz, E)
            az = headp.tile([128, TW], b16, tag="az")
            nc.vector.tensor_mul(az, pz, a_sb)
            sums = {}
            for qn, src in (("S", E), ("W", Ez), ("U", az), ("B", a_sb)):
                p = psum_s.tile([128, TW], f32, tag="ps")
                nc.tensor.matmul(p, lhsT=bdiag, rhs=src, start=True, stop=True)
                sums[qn] = p
            lnS = headp.tile([128, TW], f32, tag="lnS")
            nc.scalar.activation(lnS, sums["S"], AF.Ln)
            Sinv = headp.tile([128, TW], b16, tag="Sinv")
            nc.scalar.activation(Sinv, lnS, AF.Exp, scale=-1.0)
            t1 = headp.tile([128, TW], b16, tag="t1")
            nc.vector.tensor_mul(t1, sums["B"], lnS)
            alp = headp.tile([128, TW], b16, tag="alp")
            nc.vector.tensor_sub(alp, sums["U"], t1)
            t2 = headp.tile([128, TW], b16, tag="t2")
            nc.vector.tensor_mul(t2, sums["W"], Sinv)
            ent = headp.tile([128, TW], b16, tag="ent")
            nc.vector.tensor_sub(ent, lnS, t2)

            gstk = headp.tile([128, TW], f32, tag="gstk")
            gsrc = bass_mod.AP(tensor=gbuf[:].tensor, offset=g * GW,
                               ap=[[TW, 4], [0, 32], [1, TW]])
            nc.gpsimd.dma_start(out=gstk, in_=gsrc)
            rstk = headp.tile([128, TW], f32, tag="rstk")
            rsrc = bass_mod.AP(tensor=rwbuf[:].tensor, offset=g * GW,
                               ap=[[TW, 4], [0, 32], [1, TW]])
            nc.gpsimd.dma_start(out=rstk, in_=rsrc)
            scrap = headp.tile([128, TW], b16, tag="scrap")
            nc.vector.scalar_tensor_tensor(scrap, alp, 1.0, gstk, OP.mult, OP.mult,
                                           accum_out=pol_cols[:, g:g + 1])
            scrap2 = headp.tile([128, TW], b16, tag="scrap2")
            nc.vector.scalar_tensor_tensor(scrap2, ent, 1.0, rstk, OP.mult, OP.mult,
                                           accum_out=ent_cols[:, g:g + 1])

        pol_sum = persist.tile([128, 1], f32, tag="pol_sum")
        nc.vector.tensor_reduce(pol_sum, pol_cols, axis=mybir.AxisListType.X, op=OP.add)
        ent_sum = persist.tile([128, 1], f32, tag="ent_sum")
        nc.vector.tensor_reduce(ent_sum, ent_cols, axis=mybir.AxisListType.X, op=OP.add)
        nc.sync.dma_start(out=out_e[:, 1:2], in_=pol_sum)
        nc.sync.dma_start(out=out_e[:, 2:3], in_=ent_sum)

    import concourse.bacc as bacc_mod
    import concourse.hw_specs as hw_specs
    orig_tables = hw_specs.get_activation_tables
    keep = "natural_log_exp_and_others"
    mine = {AF.Identity, AF.Copy, AF.Exp, AF.Ln, AF.Square}

    def patched_tables(arch):
        out = {}
        for name, s in orig_tables(arch).items():
            out[name] = s if name == keep else (s - mine)
        return out

    bacc_mod.get_activation_tables = patched_tables
    try:
        nc.compile()
    finally:
        bacc_mod.get_activation_tables = orig_tables
    return nc


_NC_CACHE = {}


def kernel(features, rewards_mean, terminals_mean, actions,
           actor_params, critic_params, target_params, _want_timing=False):
    from concourse.bass_utils import run_bass_kernel_spmd

    in_maps = prep_all(features, rewards_mean, terminals_mean, actions,
                       actor_params, critic_params, target_params)
    if "nc" not in _NC_CACHE:
        _NC_CACHE["nc"] = build_graph()
    nc = _NC_CACHE["nc"]
    res = run_bass_kernel_spmd(nc, in_maps, core_ids=list(range(NCORES)),
                               trace=_want_timing)
    outs = [np.asarray(r["out"], np.float64) for r in res.results]
    val = sum(o[0:MC, 0].sum() for o in outs)
    pol = sum(o[:, 1].sum() for o in outs) / 32.0
    ent = sum(o[:, 2].sum() for o in outs) / 32.0
    denom = float(H * M)
    loss_value = 0.5 * val / denom
    loss_policy = -pol / denom
    policy_entropy = ent / denom
    loss_actor = loss_policy - TEMPERATURE * policy_entropy
    out = np.array([loss_actor, loss_value], np.float32)
    if _want_timing:
        return out, res
    return out
